# revision 9
# baseline (speedup 1.0000x reference)
"""Trainium2 8-core Bass kernel for the UniGAT hypergraph attention block.

Algorithm (matches the jax reference numerically, up to bf16 rounding):
  1. Xh = X @ theta_cat + b          (per-core node shard, PE matmul)
  2. v2e: esum[e] = sum over incidence pairs (e,v) of Xh[v]
       - per-core partial over its node shard: dma_gather of Xh rows per
         pair (sorted by edge) + 0/1-indicator segment matmul on PE
       - AllReduce(esum) over the 8 cores
  3. Softmax folding: w = exp(s)/sum(exp(s)) exactly (the segment-max
     subtraction cancels; s = leaky_relu in [-0.5, 0.5] so exp is safe).
     Build per-edge table Z = [Y*expS | expS] where Y = esum*inv_cnt,
     expS[e,h] = exp(leaky_relu(inv_cnt*(esum @ aw_h))).
  4. e2v: plain 0/1 segment-sum of gathered Z rows per destination vertex
     (sorted by vertex) -> numerator (256 cols) and denominator (4 cols);
     divide per head.
  5. ELU -> LayerNorm -> GELU -> conv matmul -> X + gamma * Xo.

Sharding: nodes (and pairs grouped by destination vertex) across 8 cores;
weights and edge tables replicated; one AllReduce of esum is the only
collective.
"""

import os

import numpy as np
import ml_dtypes

import concourse.bass as bass
import concourse.bacc as bacc
import concourse.tile as tile
import concourse.mybir as mybir
from concourse.bass_utils import run_bass_kernel_spmd
from concourse.library_config import mlp

BF16 = mybir.dt.bfloat16
F32 = mybir.dt.float32
I16 = mybir.dt.int16

P = 128
NCORES = 8

N_NODES = 100000
N_EDGES = 20000
NNZ = 500000
CH = 256  # in = hid = out channels
H = 4
DH = 64
NEG_SLOPE = 0.2
LN_EPS = 1e-6

NPC = N_NODES // NCORES          # real nodes per core (12500)
VG = (NPC + P - 1) // P          # vertex groups per core (98)
NPC_PAD = VG * P                 # padded nodes per core (12544)
EG = (N_EDGES + P - 1) // P      # edge groups (157)
E_PAD = EG * P                   # padded edges (20096)

ZW = 384                         # Z table row width (260 used, 384 for DMA-gather %256)
GATHER_CALL_V2E = 8192           # pairs per dma_gather call (64 chunks of 128)
GATHER_CALL_E2V = 4096           # pairs per dma_gather call (32 chunks of 128)


def _bf(x):
    return np.asarray(x, dtype=ml_dtypes.bfloat16)


def _wrap16(idx):
    """dma_gather index layout: index i -> [i % 16, i // 16], replicated to
    128 partitions (8 Q7 cores)."""
    assert idx.size % 16 == 0
    w = idx.reshape(-1, 16).T  # [16, n/16]
    return np.ascontiguousarray(np.tile(w, (8, 1))).astype(np.int16)


def _pairmajor(vals, dtype):
    """pair i -> [i % 128, i // 128]."""
    assert vals.size % P == 0
    return np.ascontiguousarray(vals.reshape(-1, P).T).astype(dtype)


def make_plan(edge_idx, vertex_idx):
    """Host-side index preprocessing. Returns the per-core gather/indicator
    streams (identical shapes across cores, so all cores run one SPMD graph)
    plus the shared per-group chunk counts baked into the instruction stream.
    """
    edge_idx = np.asarray(edge_idx).astype(np.int64)
    vertex_idx = np.asarray(vertex_idx).astype(np.int64)
    core = vertex_idx // NPC
    lv = vertex_idx - core * NPC

    # ---- per-core sorted pair lists ----
    v2e_e, v2e_lv = [], []   # sorted by edge
    e2v_e, e2v_lv = [], []   # sorted by local vertex
    for c in range(NCORES):
        m = core == c
        e_c, lv_c = edge_idx[m], lv[m]
        o = np.argsort(e_c, kind="stable")
        v2e_e.append(e_c[o])
        v2e_lv.append(lv_c[o])
        o = np.argsort(lv_c, kind="stable")
        e2v_e.append(e_c[o])
        e2v_lv.append(lv_c[o])

    # ---- shared chunk counts (max over cores) ----
    def group_counts(keys_list, ngroups):
        cnts = np.zeros((NCORES, ngroups), dtype=np.int64)
        for c in range(NCORES):
            cnts[c] = np.bincount(keys_list[c] // P, minlength=ngroups)
        chunks = np.maximum(1, -(-cnts.max(axis=0) // P))  # ceil, min 1
        return cnts, chunks

    v2e_cnts, v2e_chunks = group_counts(v2e_e, EG)
    e2v_cnts, e2v_chunks = group_counts(e2v_lv, VG)

    def build_streams(keys, vals, cnts, chunks, ngroups, relmod_keys):
        """keys: group source (sorted); vals: gather row index; rel from
        relmod_keys % 128. Returns (idx_streams [NCORES, T], rel_streams)."""
        T = int(chunks.sum()) * P
        idx_s = np.zeros((NCORES, T), dtype=np.int64)
        rel_s = np.full((NCORES, T), 255.0, dtype=np.float32)
        starts = np.concatenate([[0], np.cumsum(chunks)]) * P
        for c in range(NCORES):
            gstart = np.concatenate([[0], np.cumsum(cnts[c])])
            for g in range(ngroups):
                n = cnts[c][g]
                if n == 0:
                    continue
                s, d = gstart[g], starts[g]
                idx_s[c, d:d + n] = vals[c][s:s + n]
                rel_s[c, d:d + n] = relmod_keys[c][s:s + n] % P
        return idx_s, rel_s

    v2e_idx, v2e_rel = build_streams(v2e_e, v2e_lv, v2e_cnts, v2e_chunks, EG, v2e_e)
    e2v_idx, e2v_rel = build_streams(e2v_lv, e2v_e, e2v_cnts, e2v_chunks, VG, e2v_lv)

    cnt = np.bincount(edge_idx, minlength=E_PAD).astype(np.float32)
    inv_cnt = 1.0 / np.maximum(cnt, 1.0)

    return dict(
        v2e_chunks=[int(x) for x in v2e_chunks],
        e2v_chunks=[int(x) for x in e2v_chunks],
        v2e_idx=v2e_idx, v2e_rel=v2e_rel,
        e2v_idx=e2v_idx, e2v_rel=e2v_rel,
        inv_cnt=inv_cnt,
    )


def _n_gather_calls(total_chunks, call_pairs):
    total = total_chunks * P
    n_full, rem = divmod(total, call_pairs)
    sizes = [call_pairs] * n_full
    if rem:
        sizes.append(rem)
    return sizes


def build_kernel(v2e_chunks, e2v_chunks, debug_tables=False):
    v2e_tot = sum(v2e_chunks) * P
    e2v_tot = sum(e2v_chunks) * P

    nc = bacc.Bacc("TRN2", target_bir_lowering=False, debug=False,
                   num_devices=NCORES)

    # ---- I/O ----
    x_in = nc.dram_tensor("x", [NPC_PAD, CH], F32, kind="ExternalInput")
    xt_in = nc.dram_tensor("xt", [P, VG * 2 * P], BF16, kind="ExternalInput")
    wcat_in = nc.dram_tensor("wcat", [CH, CH], BF16, kind="ExternalInput")
    convw_in = nc.dram_tensor("convw", [CH, CH], BF16, kind="ExternalInput")
    brep_in = nc.dram_tensor("brep", [P, CH], F32, kind="ExternalInput")
    awrep_in = nc.dram_tensor("awrep", [P, CH], BF16, kind="ExternalInput")
    convbrep_in = nc.dram_tensor("convbrep", [P, CH], F32, kind="ExternalInput")
    gammarep_in = nc.dram_tensor("gammarep", [P, CH], F32, kind="ExternalInput")
    lnwrep_in = nc.dram_tensor("lnwrep", [P, CH], F32, kind="ExternalInput")
    lnbrep_in = nc.dram_tensor("lnbrep", [P, CH], F32, kind="ExternalInput")
    iota_in = nc.dram_tensor("iota", [P, P], BF16, kind="ExternalInput")
    ident_in = nc.dram_tensor("ident", [P, P], BF16, kind="ExternalInput")
    invc_in = nc.dram_tensor("invc", [P, EG], F32, kind="ExternalInput")
    v2ei_in = nc.dram_tensor("v2ei", [P, v2e_tot // 16], I16, kind="ExternalInput")
    v2er_in = nc.dram_tensor("v2er", [P, v2e_tot // P], BF16, kind="ExternalInput")
    e2vi_in = nc.dram_tensor("e2vi", [P, e2v_tot // 16], I16, kind="ExternalInput")
    e2vr_in = nc.dram_tensor("e2vr", [P, e2v_tot // P], BF16, kind="ExternalInput")
    out_ext = nc.dram_tensor("out", [NPC_PAD, CH], F32, kind="ExternalOutput")
    dbg = {}
    if debug_tables:
        dbg["xh"] = nc.dram_tensor("dbg_xh", [NPC_PAD, CH], BF16, kind="ExternalOutput")
        dbg["esum"] = nc.dram_tensor("dbg_esum", [E_PAD, CH], BF16, kind="ExternalOutput")
        dbg["z"] = nc.dram_tensor("dbg_z", [E_PAD, ZW], BF16, kind="ExternalOutput")
        dbg["xn"] = nc.dram_tensor("dbg_xn", [NPC_PAD, CH], BF16, kind="ExternalOutput")

    with tile.TileContext(nc) as tc:
        with tc.tile_pool(name="dram", bufs=1, space="DRAM") as dram, \
             tc.tile_pool(name="const", bufs=1) as cpool, \
             tc.tile_pool(name="resident", bufs=1) as rpool:

            nc.gpsimd.load_library(mlp)

            xh_table = dram.tile([NPC_PAD, CH], BF16)
            esum_bounce = dram.tile([E_PAD, CH], BF16)
            yfull = dram.tile([E_PAD, CH], BF16, addr_space="Shared")
            z_table = dram.tile([E_PAD, ZW], BF16)

            # ---- resident constants ----
            def cload(dr, shape, dtype, name):
                t = cpool.tile(shape, dtype, name=name, tag=name)
                nc.sync.dma_start(t[:], dr[:])
                return t

            w_sb = cpool.tile([P, 2, CH], BF16)
            nc.sync.dma_start(w_sb[:], wcat_in[:].rearrange("(k p) f -> p k f", p=P))
            convw_sb = cpool.tile([P, 2, CH], BF16)
            nc.sync.dma_start(convw_sb[:], convw_in[:].rearrange("(k p) f -> p k f", p=P))
            brep = cload(brep_in, [P, CH], F32, "brep")
            awrep = cload(awrep_in, [P, CH], BF16, "awrep")
            convbrep = cload(convbrep_in, [P, CH], F32, "convbrep")
            gammarep = cload(gammarep_in, [P, CH], F32, "gammarep")
            lnwrep = cload(lnwrep_in, [P, CH], F32, "lnwrep")
            lnbrep = cload(lnbrep_in, [P, CH], F32, "lnbrep")
            iota = cload(iota_in, [P, P], BF16, "iota")
            ident = cload(ident_in, [P, P], BF16, "ident")
            invc = cload(invc_in, [P, EG], F32, "invc")

            xn_sb = rpool.tile([P, VG, CH], BF16)

            # ================= Phase 1: Xh = X @ W + b =================
            with tc.tile_pool(name="p1sb", bufs=3) as p1sb, \
                 tc.tile_pool(name="p1ps", bufs=2, space="PSUM") as p1ps:
                xt_sb = rpool.tile([P, VG * 2 * P], BF16)
                nc.sync.dma_start(xt_sb[:], xt_in[:])
                xt_v = xt_sb[:].rearrange("p (t k f) -> p t k f", t=VG, k=2)
                for t in range(VG):
                    psf = p1ps.tile([P, 512], F32, tag="xhps")
                    ps = psf[:, :CH]
                    for k in range(2):
                        nc.tensor.matmul(ps, lhsT=xt_v[:, t, k, :], rhs=w_sb[:, k, :],
                                         start=(k == 0), stop=(k == 1))
                    xh = p1sb.tile([P, CH], BF16, tag="xhout")
                    nc.vector.tensor_tensor(out=xh[:], in0=ps, in1=brep[:],
                                            op=mybir.AluOpType.add)
                    nc.sync.dma_start(xh_table[t * P:(t + 1) * P, :], xh[:])
                if debug_tables:
                    nc.sync.dma_start(dbg["xh"][:], xh_table[:])

            # ================= Phase 2: v2e partial esum =================
            with tc.tile_pool(name="v2esb", bufs=2) as gpool, \
                 tc.tile_pool(name="v2esel", bufs=4) as selpool, \
                 tc.tile_pool(name="v2eev", bufs=3) as evpool, \
                 tc.tile_pool(name="v2eidx", bufs=1) as ipool, \
                 tc.tile_pool(name="v2eps", bufs=3, space="PSUM") as v2eps:
                v2ei = ipool.tile([P, v2e_tot // 16], I16)
                nc.sync.dma_start(v2ei[:], v2ei_in[:])
                v2er = ipool.tile([P, v2e_tot // P], BF16)
                nc.sync.dma_start(v2er[:], v2er_in[:])

                call_sizes = _n_gather_calls(sum(v2e_chunks), GATHER_CALL_V2E)
                gtiles = [None] * len(call_sizes)
                mm = 0
                for g in range(EG):
                    psf = v2eps.tile([P, 512], F32, tag="v2eps")
                    ps = psf[:, :CH]
                    for k in range(v2e_chunks[g]):
                        gc, j = divmod(mm, GATHER_CALL_V2E // P)
                        if gtiles[gc] is None:
                            n = call_sizes[gc]
                            gt = gpool.tile([P, GATHER_CALL_V2E // P, CH], BF16,
                                            tag="v2egather")
                            s = gc * GATHER_CALL_V2E
                            nc.gpsimd.dma_gather(
                                gt[:, :n // P, :], xh_table[:],
                                v2ei[:, s // 16:(s + n) // 16], n, n, CH,
                                single_packet=False)
                            gtiles[gc] = gt
                        selT = selpool.tile([P, P], BF16, tag="v2esel")
                        nc.vector.tensor_tensor(
                            out=selT[:],
                            in0=v2er[:, mm:mm + 1].to_broadcast([P, P]),
                            in1=iota[:], op=mybir.AluOpType.is_equal)
                        nc.tensor.matmul(ps, lhsT=selT[:], rhs=gtiles[gc][:, j, :],
                                         start=(k == 0), stop=(k == v2e_chunks[g] - 1))
                        mm += 1
                    esb = evpool.tile([P, CH], BF16, tag="v2eev")
                    nc.vector.tensor_copy(out=esb[:], in_=ps)
                    nc.sync.dma_start(esum_bounce[g * P:(g + 1) * P, :], esb[:])

            # ================= Phase 3: AllReduce =================
            nc.gpsimd.collective_compute(
                "AllReduce", mybir.AluOpType.add,
                replica_groups=[list(range(NCORES))],
                ins=[esum_bounce.opt()], outs=[yfull.opt()])
            if debug_tables:
                nc.sync.dma_start(dbg["esum"][:], yfull[:])

            # ================= Phase 4: Z table =================
            with tc.tile_pool(name="zsb", bufs=3) as zpool:
                for t in range(EG):
                    y = zpool.tile([P, CH], BF16, tag="zy")
                    nc.sync.dma_start(y[:], yfull[t * P:(t + 1) * P, :])
                    tmp = zpool.tile([P, CH], BF16, tag="ztmp")
                    nc.vector.tensor_tensor(out=tmp[:], in0=y[:], in1=awrep[:],
                                            op=mybir.AluOpType.mult)
                    beta = zpool.tile([P, H], F32, tag="zbeta")
                    nc.vector.tensor_reduce(
                        out=beta[:], in_=tmp[:].rearrange("p (h d) -> p h d", d=DH),
                        axis=mybir.AxisListType.X, op=mybir.AluOpType.add)
                    # alpha = beta * inv_cnt ; salpha = leaky_relu(alpha)
                    al = zpool.tile([P, H], F32, tag="zal")
                    nc.vector.tensor_scalar_mul(al[:], beta[:], invc[:, t:t + 1])
                    al2 = zpool.tile([P, H], F32, tag="zal2")
                    nc.vector.tensor_scalar_mul(al2[:], al[:], NEG_SLOPE)
                    sal = zpool.tile([P, H], F32, tag="zsal")
                    nc.vector.tensor_tensor(out=sal[:], in0=al[:], in1=al2[:],
                                            op=mybir.AluOpType.max)
                    zrow = zpool.tile([P, CH + H], BF16, tag="zrow")
                    expS = zrow[:, CH:CH + H]
                    nc.scalar.activation(out=expS, in_=sal[:],
                                         func=mybir.ActivationFunctionType.Exp)
                    # s4 = expS * inv_cnt ; Z = esum * s4 (broadcast 64)
                    s4 = zpool.tile([P, H], F32, tag="zs4")
                    nc.vector.tensor_scalar_mul(s4[:], expS, invc[:, t:t + 1])
                    nc.vector.tensor_tensor(
                        out=zrow[:, :CH].rearrange("p (h d) -> p h d", d=DH),
                        in0=y[:].rearrange("p (h d) -> p h d", d=DH),
                        in1=s4[:, :, None].to_broadcast([P, H, DH]),
                        op=mybir.AluOpType.mult)
                    nc.sync.dma_start(z_table[t * P:(t + 1) * P, :CH + H], zrow[:])
                if debug_tables:
                    nc.sync.dma_start(dbg["z"][:], z_table[:])

            # ================= Phase 5: e2v + ELU + LN =================
            with tc.tile_pool(name="e2vsb", bufs=2) as gpool2, \
                 tc.tile_pool(name="e2vsel", bufs=4) as selpool2, \
                 tc.tile_pool(name="e2vev", bufs=3) as evpool2, \
                 tc.tile_pool(name="e2vidx", bufs=1) as ipool2, \
                 tc.tile_pool(name="e2vps", bufs=3, space="PSUM") as e2vps:
                e2vi = ipool2.tile([P, e2v_tot // 16], I16)
                nc.sync.dma_start(e2vi[:], e2vi_in[:])
                e2vr = ipool2.tile([P, e2v_tot // P], BF16)
                nc.sync.dma_start(e2vr[:], e2vr_in[:])

                call_sizes = _n_gather_calls(sum(e2v_chunks), GATHER_CALL_E2V)
                gtiles = [None] * len(call_sizes)
                mm = 0
                for g in range(VG):
                    psf = e2vps.tile([P, 512], F32, tag="e2vps")
                    ps = psf[:, :CH + H]
                    for k in range(e2v_chunks[g]):
                        gc, j = divmod(mm, GATHER_CALL_E2V // P)
                        if gtiles[gc] is None:
                            n = call_sizes[gc]
                            gt = gpool2.tile([P, GATHER_CALL_E2V // P, ZW], BF16,
                                             tag="e2vgather")
                            s = gc * GATHER_CALL_E2V
                            nc.gpsimd.dma_gather(
                                gt[:, :n // P, :], z_table[:],
                                e2vi[:, s // 16:(s + n) // 16], n, n, ZW,
                                single_packet=False)
                            gtiles[gc] = gt
                        selT = selpool2.tile([P, P], BF16, tag="e2vsel")
                        nc.vector.tensor_tensor(
                            out=selT[:],
                            in0=e2vr[:, mm:mm + 1].to_broadcast([P, P]),
                            in1=iota[:], op=mybir.AluOpType.is_equal)
                        nc.tensor.matmul(ps, lhsT=selT[:],
                                         rhs=gtiles[gc][:, j, :CH + H],
                                         start=(k == 0), stop=(k == e2v_chunks[g] - 1))
                        mm += 1
                    # divide by den, ELU, LayerNorm
                    den = evpool2.tile([P, H], F32, tag="den")
                    nc.vector.tensor_scalar_max(den[:], ps[:, CH:CH + H], 1e-12)
                    rec = evpool2.tile([P, H], F32, tag="rec")
                    nc.vector.reciprocal(rec[:], den[:])
                    xpre = evpool2.tile([P, CH], BF16, tag="xpre")
                    nc.vector.tensor_tensor(
                        out=xpre[:].rearrange("p (h d) -> p h d", d=DH),
                        in0=ps[:, :CH].rearrange("p (h d) -> p h d", d=DH),
                        in1=rec[:, :, None].to_broadcast([P, H, DH]),
                        op=mybir.AluOpType.mult)
                    # ELU = exp(min(x,0)) - 1 + x - min(x,0)
                    m0 = evpool2.tile([P, CH], BF16, tag="m0")
                    nc.vector.tensor_scalar_min(m0[:], xpre[:], 0.0)
                    ep = evpool2.tile([P, CH], F32, tag="ep")
                    nc.scalar.activation(out=ep[:], in_=m0[:],
                                         func=mybir.ActivationFunctionType.Exp)
                    t1 = evpool2.tile([P, CH], F32, tag="t1")
                    nc.vector.scalar_tensor_tensor(
                        out=t1[:], in0=ep[:], scalar=-1.0, in1=xpre[:],
                        op0=mybir.AluOpType.add, op1=mybir.AluOpType.add)
                    elu = evpool2.tile([P, CH], F32, tag="elu")
                    s1 = evpool2.tile([P, 1], F32, tag="s1")
                    nc.vector.scalar_tensor_tensor(
                        out=elu[:], in0=t1[:], scalar=0.0, in1=m0[:],
                        op0=mybir.AluOpType.add, op1=mybir.AluOpType.subtract,
                        accum_out=s1[:])
                    mu = evpool2.tile([P, 1], F32, tag="mu")
                    nc.vector.tensor_scalar_mul(mu[:], s1[:], 1.0 / CH)
                    xc = evpool2.tile([P, CH], F32, tag="xc")
                    nc.vector.tensor_scalar_sub(xc[:], elu[:], mu[:])
                    sq = evpool2.tile([P, CH], F32, tag="sq")
                    ss = evpool2.tile([P, 1], F32, tag="ss")
                    nc.vector.scalar_tensor_tensor(
                        out=sq[:], in0=xc[:], scalar=1.0, in1=xc[:],
                        op0=mybir.AluOpType.mult, op1=mybir.AluOpType.mult,
                        accum_out=ss[:])
                    var = evpool2.tile([P, 1], F32, tag="var")
                    nc.vector.tensor_scalar(var[:], ss[:], 1.0 / CH, LN_EPS,
                                            mybir.AluOpType.mult,
                                            mybir.AluOpType.add)
                    lnv = evpool2.tile([P, 1], F32, tag="lnv")
                    nc.scalar.activation(out=lnv[:], in_=var[:],
                                         func=mybir.ActivationFunctionType.Ln)
                    rstd = evpool2.tile([P, 1], F32, tag="rstd")
                    nc.scalar.activation(out=rstd[:], in_=lnv[:],
                                         func=mybir.ActivationFunctionType.Exp,
                                         scale=-0.5)
                    xn1 = evpool2.tile([P, CH], F32, tag="xn1")
                    nc.vector.scalar_tensor_tensor(
                        out=xn1[:], in0=xc[:], scalar=rstd[:], in1=lnwrep[:],
                        op0=mybir.AluOpType.mult, op1=mybir.AluOpType.mult)
                    nc.vector.tensor_tensor(out=xn_sb[:, g, :], in0=xn1[:],
                                            in1=lnbrep[:], op=mybir.AluOpType.add)

            # ================= Phase 6: GELU + conv + residual =================
            with tc.tile_pool(name="fsb", bufs=3) as fpool, \
                 tc.tile_pool(name="fps", bufs=2, space="PSUM") as fps, \
                 tc.tile_pool(name="ftps", bufs=2, space="PSUM") as ftps:
                if debug_tables:
                    nc.sync.dma_start(dbg["xn"][:].rearrange("(t p) f -> p t f", p=P), xn_sb[:])
                for g in range(VG):
                    xg = fpool.tile([P, CH], BF16, tag="xg")
                    nc.scalar.activation(out=xg[:], in_=xn_sb[:, g, :],
                                         func=mybir.ActivationFunctionType.Gelu)
                    xgT = fpool.tile([P, 2, P], BF16, tag="xgT")
                    for k in range(2):
                        tp = ftps.tile([P, P], BF16, tag="tps")
                        nc.tensor.transpose(tp[:], xg[:, k * P:(k + 1) * P], ident[:])
                        nc.scalar.copy(out=xgT[:, k, :], in_=tp[:])
                    psf = fps.tile([P, 512], F32, tag="fps")
                    ps = psf[:, :CH]
                    for k in range(2):
                        nc.tensor.matmul(ps, lhsT=xgT[:, k, :], rhs=convw_sb[:, k, :],
                                         start=(k == 0), stop=(k == 1))
                    xo = fpool.tile([P, CH], F32, tag="xo")
                    nc.vector.tensor_tensor(out=xo[:], in0=ps, in1=convbrep[:],
                                            op=mybir.AluOpType.add)
                    xl = fpool.tile([P, CH], F32, tag="xl")
                    nc.sync.dma_start(xl[:], x_in[g * P:(g + 1) * P, :])
                    xo2 = fpool.tile([P, CH], F32, tag="xo2")
                    nc.vector.tensor_tensor(out=xo2[:], in0=xo[:], in1=gammarep[:],
                                            op=mybir.AluOpType.mult)
                    ofin = fpool.tile([P, CH], F32, tag="ofin")
                    nc.vector.tensor_tensor(out=ofin[:], in0=xo2[:], in1=xl[:],
                                            op=mybir.AluOpType.add)
                    nc.sync.dma_start(out_ext[g * P:(g + 1) * P, :], ofin[:])

    nc.compile()
    return nc


def prepare_inputs(X, edge_idx, vertex_idx, theta_w, theta_b, atten_w,
                   ln_w, ln_b, conv_w, conv_b, gamma, plan):
    """Build the 8 per-core input maps."""
    X = np.asarray(X, dtype=np.float32)
    theta_w = np.asarray(theta_w, dtype=np.float32)
    wcat = _bf(theta_w.transpose(1, 0, 2).reshape(CH, CH))
    brep = np.tile(np.asarray(theta_b, np.float32).reshape(1, CH), (P, 1))
    awrep = _bf(np.tile(np.asarray(atten_w, np.float32).reshape(1, CH), (P, 1)))
    convw = _bf(np.asarray(conv_w, np.float32))
    convbrep = np.tile(np.asarray(conv_b, np.float32).reshape(1, CH), (P, 1))
    gammarep = np.tile(np.asarray(gamma, np.float32).reshape(1, CH), (P, 1))
    lnwrep = np.tile(np.asarray(ln_w, np.float32).reshape(1, CH), (P, 1))
    lnbrep = np.tile(np.asarray(ln_b, np.float32).reshape(1, CH), (P, 1))
    iota = _bf(np.tile(np.arange(P, dtype=np.float32), (P, 1)))
    ident = _bf(np.eye(P, dtype=np.float32))
    invc = np.ascontiguousarray(
        plan["inv_cnt"].reshape(EG, P).T).astype(np.float32)

    in_maps = []
    for c in range(NCORES):
        xc = np.zeros((NPC_PAD, CH), np.float32)
        xc[:NPC] = X[c * NPC:(c + 1) * NPC]
        xcb = _bf(xc)
        # xt: [128, VG, 2, 128] with xt[p, t, k, j] = x[t*128+j, k*128+p]
        xt = np.ascontiguousarray(
            xcb.reshape(VG, P, 2, P).transpose(3, 0, 2, 1)).reshape(P, VG * 2 * P)
        in_maps.append(dict(
            x=xc, xt=_bf(xt), wcat=wcat, convw=convw,
            brep=brep.astype(np.float32), awrep=awrep,
            convbrep=convbrep.astype(np.float32),
            gammarep=gammarep.astype(np.float32),
            lnwrep=lnwrep.astype(np.float32), lnbrep=lnbrep.astype(np.float32),
            iota=iota, ident=ident, invc=invc,
            v2ei=_wrap16(plan["v2e_idx"][c]),
            v2er=_pairmajor(plan["v2e_rel"][c], ml_dtypes.bfloat16),
            e2vi=_wrap16(plan["e2v_idx"][c]),
            e2vr=_pairmajor(plan["e2v_rel"][c], ml_dtypes.bfloat16),
        ))
    return in_maps


_CACHE = {}


def kernel(X, edge_idx, vertex_idx, theta_w, theta_b, atten_w,
           ln_w, ln_b, conv_w, conv_b, gamma):
    debug_tables = bool(int(os.environ.get("GNN_DEBUG_TABLES", "0")))
    trace = bool(int(os.environ.get("GNN_TRACE", "0")))

    plan = make_plan(edge_idx, vertex_idx)
    key = (tuple(plan["v2e_chunks"]), tuple(plan["e2v_chunks"]), debug_tables)
    if key not in _CACHE:
        _CACHE[key] = build_kernel(plan["v2e_chunks"], plan["e2v_chunks"],
                                   debug_tables=debug_tables)
    nc = _CACHE[key]

    in_maps = prepare_inputs(X, edge_idx, vertex_idx, theta_w, theta_b,
                             atten_w, ln_w, ln_b, conv_w, conv_b, gamma, plan)
    res = run_bass_kernel_spmd(nc, in_maps, core_ids=list(range(NCORES)),
                               trace=trace)
    kernel.last_results = res
    out = np.concatenate(
        [np.asarray(res.results[c]["out"])[:NPC] for c in range(NCORES)], axis=0)
    return out.astype(np.float32)


# revision 11
# speedup vs baseline: 1.5228x; 1.5228x over previous
"""Trainium2 8-core Bass kernel for the UniGAT hypergraph attention block.

Algorithm (matches the jax reference numerically, up to bf16 rounding):
  1. Xh = X @ theta_cat + b          (per-core node shard, PE matmul)
  2. v2e: esum[e] = sum over incidence pairs (e,v) of Xh[v]
       - per-core partial over its node shard: dma_gather of Xh rows per
         pair (sorted by edge) + 0/1-indicator segment matmul on PE
       - AllReduce(esum) over the 8 cores
  3. Softmax folding: w = exp(s)/sum(exp(s)) exactly (the segment-max
     subtraction cancels; s = leaky_relu in [-0.5, 0.5] so exp is safe).
     Build per-edge table Z = [Y*expS | expS] where Y = esum*inv_cnt,
     expS[e,h] = exp(leaky_relu(inv_cnt*(esum @ aw_h))).
  4. e2v: plain 0/1 segment-sum of gathered Z rows per destination vertex
     (sorted by vertex) -> numerator (256 cols) and denominator (4 cols);
     divide per head.
  5. ELU -> LayerNorm -> GELU -> conv matmul -> X + gamma * Xo.

Sharding: nodes (and pairs grouped by destination vertex) across 8 cores;
weights and edge tables replicated; one AllReduce of esum is the only
collective.
"""

import os

import numpy as np
import ml_dtypes

import concourse.bass as bass
import concourse.bacc as bacc
import concourse.tile as tile
import concourse.mybir as mybir
from concourse.bass_utils import run_bass_kernel_spmd
from concourse.library_config import mlp

BF16 = mybir.dt.bfloat16
F32 = mybir.dt.float32
I16 = mybir.dt.int16
AL = mybir.AluOpType
AF = mybir.ActivationFunctionType

P = 128
NCORES = 8

N_NODES = 100000
N_EDGES = 20000
NNZ = 500000
CH = 256
H = 4
DH = 64
NEG_SLOPE = 0.2
LN_EPS = 1e-6

NPC = N_NODES // NCORES          # 12500
VG = (NPC + P - 1) // P          # 98
NPC_PAD = VG * P                 # 12544
EG = (N_EDGES + P - 1) // P      # 157
E_PAD = EG * P                   # 20096

ZW = 384                         # Z table row stride (260 used; %128 elems)
GATHER_CALL_V2E = 8192
GATHER_CALL_E2V = 4096
SELW = 8                         # indicator chunks built per DVE op


def _bf(x):
    return np.asarray(x, dtype=ml_dtypes.bfloat16)


def _wrap16(idx):
    """dma_gather index layout: index i -> [i % 16, i // 16], replicated x8."""
    assert idx.size % 16 == 0
    w = idx.reshape(-1, 16).T
    return np.ascontiguousarray(np.tile(w, (8, 1))).astype(np.int16)


def _pairmajor(vals, dtype):
    """pair i -> [i % 128, i // 128]."""
    assert vals.size % P == 0
    return np.ascontiguousarray(vals.reshape(-1, P).T).astype(dtype)


def make_plan(edge_idx, vertex_idx):
    """Host-side index preprocessing (graph structure only)."""
    edge_idx = np.asarray(edge_idx).astype(np.int64)
    vertex_idx = np.asarray(vertex_idx).astype(np.int64)
    core = vertex_idx // NPC
    lv = vertex_idx - core * NPC

    v2e_e, v2e_lv = [], []
    e2v_e, e2v_lv = [], []
    for c in range(NCORES):
        m = core == c
        e_c, lv_c = edge_idx[m], lv[m]
        o = np.argsort(e_c, kind="stable")
        v2e_e.append(e_c[o])
        v2e_lv.append(lv_c[o])
        o = np.argsort(lv_c, kind="stable")
        e2v_e.append(e_c[o])
        e2v_lv.append(lv_c[o])

    def group_counts(keys_list, ngroups):
        cnts = np.zeros((NCORES, ngroups), dtype=np.int64)
        for c in range(NCORES):
            cnts[c] = np.bincount(keys_list[c] // P, minlength=ngroups)
        chunks = np.maximum(1, -(-cnts.max(axis=0) // P))
        return cnts, chunks

    v2e_cnts, v2e_chunks = group_counts(v2e_e, EG)
    e2v_cnts, e2v_chunks = group_counts(e2v_lv, VG)

    def build_streams(vals, cnts, chunks, ngroups, relmod_keys):
        T = int(chunks.sum()) * P
        idx_s = np.zeros((NCORES, T), dtype=np.int64)
        rel_s = np.full((NCORES, T), 255.0, dtype=np.float32)
        starts = np.concatenate([[0], np.cumsum(chunks)]) * P
        for c in range(NCORES):
            gstart = np.concatenate([[0], np.cumsum(cnts[c])])
            for g in range(ngroups):
                n = cnts[c][g]
                if n == 0:
                    continue
                s, d = gstart[g], starts[g]
                idx_s[c, d:d + n] = vals[c][s:s + n]
                rel_s[c, d:d + n] = relmod_keys[c][s:s + n] % P
        return idx_s, rel_s

    v2e_idx, v2e_rel = build_streams(v2e_lv, v2e_cnts, v2e_chunks, EG, v2e_e)
    e2v_idx, e2v_rel = build_streams(e2v_e, e2v_cnts, e2v_chunks, VG, e2v_lv)

    cnt = np.bincount(edge_idx, minlength=E_PAD).astype(np.float32)
    inv_cnt = 1.0 / np.maximum(cnt, 1.0)

    return dict(
        v2e_chunks=[int(x) for x in v2e_chunks],
        e2v_chunks=[int(x) for x in e2v_chunks],
        v2e_idx=v2e_idx, v2e_rel=v2e_rel,
        e2v_idx=e2v_idx, e2v_rel=e2v_rel,
        inv_cnt=inv_cnt,
    )


def _n_gather_calls(total_chunks, call_pairs):
    total = total_chunks * P
    n_full, rem = divmod(total, call_pairs)
    sizes = [call_pairs] * n_full
    if rem:
        sizes.append(rem)
    return sizes


def build_kernel(v2e_chunks, e2v_chunks, debug_tables=False):
    v2e_tot = sum(v2e_chunks) * P
    e2v_tot = sum(e2v_chunks) * P

    nc = bacc.Bacc("TRN2", target_bir_lowering=False, debug=False,
                   num_devices=NCORES, num_swdge_queues=2,
                   dynamic_dma_scratch_size=32768)

    x_in = nc.dram_tensor("x", [NPC_PAD, CH], F32, kind="ExternalInput")
    xt_in = nc.dram_tensor("xt", [P, VG * 2 * P], BF16, kind="ExternalInput")
    wcat_in = nc.dram_tensor("wcat", [CH, CH], BF16, kind="ExternalInput")
    convw_in = nc.dram_tensor("convw", [CH, CH], BF16, kind="ExternalInput")
    brep_in = nc.dram_tensor("brep", [P, CH], F32, kind="ExternalInput")
    awrep_in = nc.dram_tensor("awrep", [P, CH], BF16, kind="ExternalInput")
    convbrep_in = nc.dram_tensor("convbrep", [P, CH], F32, kind="ExternalInput")
    gammarep_in = nc.dram_tensor("gammarep", [P, CH], F32, kind="ExternalInput")
    lnwrep_in = nc.dram_tensor("lnwrep", [P, CH], F32, kind="ExternalInput")
    lnbrep_in = nc.dram_tensor("lnbrep", [P, CH], F32, kind="ExternalInput")
    iota_in = nc.dram_tensor("iota", [P, P], BF16, kind="ExternalInput")
    ident_in = nc.dram_tensor("ident", [P, P], BF16, kind="ExternalInput")
    invc_in = nc.dram_tensor("invc", [P, EG], F32, kind="ExternalInput")
    v2ei_in = nc.dram_tensor("v2ei", [P, v2e_tot // 16], I16, kind="ExternalInput")
    v2er_in = nc.dram_tensor("v2er", [P, v2e_tot // P], BF16, kind="ExternalInput")
    e2vi_in = nc.dram_tensor("e2vi", [P, e2v_tot // 16], I16, kind="ExternalInput")
    e2vr_in = nc.dram_tensor("e2vr", [P, e2v_tot // P], BF16, kind="ExternalInput")
    out_ext = nc.dram_tensor("out", [NPC_PAD, CH], F32, kind="ExternalOutput")
    dbg = {}
    if debug_tables:
        dbg["xh"] = nc.dram_tensor("dbg_xh", [NPC_PAD, CH], BF16, kind="ExternalOutput")
        dbg["esum"] = nc.dram_tensor("dbg_esum", [E_PAD, CH], BF16, kind="ExternalOutput")
        dbg["z"] = nc.dram_tensor("dbg_z", [E_PAD, ZW], BF16, kind="ExternalOutput")
        dbg["xn"] = nc.dram_tensor("dbg_xn", [NPC_PAD, CH], BF16, kind="ExternalOutput")

    def rows(dr, t0, w):
        return dr[t0 * P:(t0 + w) * P, :].rearrange("(t p) f -> p t f", p=P)

    with tile.TileContext(nc) as tc:
        with tc.tile_pool(name="dram", bufs=1, space="DRAM") as dram, \
             tc.tile_pool(name="const", bufs=1) as cpool, \
             tc.tile_pool(name="resident", bufs=1) as rpool:

            nc.gpsimd.load_library(mlp)

            xh_table = dram.tile([NPC_PAD, CH], BF16)
            esum_bounce = dram.tile([E_PAD, CH], BF16)
            yfull = dram.tile([E_PAD, CH], BF16, addr_space="Shared")
            z_table = dram.tile([E_PAD, ZW], BF16)

            def cload(dr, shape, dtype, name):
                t = cpool.tile(shape, dtype, name=name, tag=name)
                nc.sync.dma_start(t[:], dr[:])
                return t

            w_sb = cpool.tile([P, 2, CH], BF16)
            nc.sync.dma_start(w_sb[:], wcat_in[:].rearrange("(k p) f -> p k f", p=P))
            convw_sb = cpool.tile([P, 2, CH], BF16)
            nc.sync.dma_start(convw_sb[:], convw_in[:].rearrange("(k p) f -> p k f", p=P))
            brep = cload(brep_in, [P, CH], F32, "brep")
            awrep = cload(awrep_in, [P, CH], BF16, "awrep")
            convbrep = cload(convbrep_in, [P, CH], F32, "convbrep")
            gammarep = cload(gammarep_in, [P, CH], F32, "gammarep")
            lnwrep = cload(lnwrep_in, [P, CH], F32, "lnwrep")
            lnbrep = cload(lnbrep_in, [P, CH], F32, "lnbrep")
            iota = cload(iota_in, [P, P], BF16, "iota")
            ident = cload(ident_in, [P, P], BF16, "ident")
            invc = cload(invc_in, [P, EG], F32, "invc")
            gcb = cpool.tile([P, CH], F32)
            nc.vector.tensor_tensor(out=gcb[:], in0=gammarep[:], in1=convbrep[:],
                                    op=AL.mult)

            xn_sb = rpool.tile([P, VG, CH], BF16)

            # ================= Phase 1: Xh = X @ W + b =================
            with tc.tile_pool(name="p1sb", bufs=3) as p1sb, \
                 tc.tile_pool(name="p1xt", bufs=1) as p1xt, \
                 tc.tile_pool(name="p1ps", bufs=2, space="PSUM") as p1ps:
                xt_sb = p1xt.tile([P, VG * 2 * P], BF16, tag="xt")
                nc.sync.dma_start(xt_sb[:], xt_in[:])
                xt_v = xt_sb[:].rearrange("p (t k f) -> p t k f", t=VG, k=2)
                xh4, t0, tw = None, 0, 0
                for t in range(VG):
                    psf = p1ps.tile([P, 512], F32, tag="xhps")
                    ps = psf[:, :CH]
                    for k in range(2):
                        nc.tensor.matmul(ps, lhsT=xt_v[:, t, k, :], rhs=w_sb[:, k, :],
                                         start=(k == 0), stop=(k == 1))
                    if t % 4 == 0:
                        t0 = t
                        tw = min(4, VG - t0)
                        xh4 = p1sb.tile([P, 4, CH], BF16, tag="xhout")
                    nc.vector.tensor_tensor(out=xh4[:, t - t0, :], in0=ps,
                                            in1=brep[:], op=AL.add)
                    if t - t0 == tw - 1:
                        nc.sync.dma_start(rows(xh_table, t0, tw), xh4[:, :tw, :])
                if debug_tables:
                    nc.sync.dma_start(dbg["xh"][:], xh_table[:])

            # ================= Phase 2: v2e partial esum =================
            with tc.tile_pool(name="v2esb", bufs=2) as gpool, \
                 tc.tile_pool(name="v2esel", bufs=3) as selpool, \
                 tc.tile_pool(name="v2eev", bufs=3) as evpool, \
                 tc.tile_pool(name="v2eidx", bufs=1) as ipool, \
                 tc.tile_pool(name="v2eps", bufs=3, space="PSUM") as v2eps:
                v2ei = ipool.tile([P, v2e_tot // 16], I16)
                nc.sync.dma_start(v2ei[:], v2ei_in[:])
                v2er = ipool.tile([P, v2e_tot // P], BF16)
                nc.sync.dma_start(v2er[:], v2er_in[:])

                call_sizes = _n_gather_calls(sum(v2e_chunks), GATHER_CALL_V2E)
                gtiles = [None] * len(call_sizes)
                tot = sum(v2e_chunks)
                sel_cur, sel0 = None, 0
                esb4, e0, ew = None, 0, 0
                mm = 0
                for g in range(EG):
                    psf = v2eps.tile([P, 512], F32, tag="v2eps")
                    ps = psf[:, :CH]
                    for k in range(v2e_chunks[g]):
                        gc, j = divmod(mm, GATHER_CALL_V2E // P)
                        if gtiles[gc] is None:
                            n = call_sizes[gc]
                            gt = gpool.tile([P, GATHER_CALL_V2E // P, CH], BF16,
                                            tag="v2egather")
                            s = gc * GATHER_CALL_V2E
                            nc.gpsimd.dma_gather(
                                gt[:, :n // P, :], xh_table[:],
                                v2ei[:, s // 16:(s + n) // 16], n, n, CH,
                                single_packet=False, queue_num=gc % 2)
                            gtiles[gc] = gt
                        if mm % SELW == 0:
                            sel0 = mm
                            sw = min(SELW, tot - mm)
                            sel_cur = selpool.tile([P, SELW, P], BF16, tag="v2esel")
                            nc.vector.tensor_tensor(
                                out=sel_cur[:, :sw, :],
                                in0=v2er[:, mm:mm + sw, None].to_broadcast([P, sw, P]),
                                in1=iota[:, None, :].to_broadcast([P, sw, P]),
                                op=AL.is_equal)
                        nc.tensor.matmul(ps, lhsT=sel_cur[:, mm - sel0, :],
                                         rhs=gtiles[gc][:, j, :],
                                         start=(k == 0), stop=(k == v2e_chunks[g] - 1))
                        mm += 1
                    if g % 4 == 0:
                        e0 = g
                        ew = min(4, EG - e0)
                        esb4 = evpool.tile([P, 4, CH], BF16, tag="v2eev")
                    nc.vector.tensor_copy(out=esb4[:, g - e0, :], in_=ps)
                    if g - e0 == ew - 1:
                        nc.sync.dma_start(rows(esum_bounce, e0, ew), esb4[:, :ew, :])

            # ================= Phase 3: AllReduce =================
            nc.gpsimd.collective_compute(
                "AllReduce", AL.add,
                replica_groups=[list(range(NCORES))],
                ins=[esum_bounce.opt()], outs=[yfull.opt()])
            if debug_tables:
                nc.sync.dma_start(dbg["esum"][:], yfull[:])

            # ================= Phase 4: Z table (4 edge tiles per iter) ====
            with tc.tile_pool(name="zsb", bufs=3) as zpool:
                for t0 in range(0, EG, 4):
                    w = min(4, EG - t0)
                    y4 = zpool.tile([P, 4, CH], BF16, tag="zy")
                    nc.sync.dma_start(y4[:, :w, :], rows(yfull, t0, w))
                    tmp = zpool.tile([P, 4, CH], BF16, tag="ztmp")
                    nc.vector.tensor_tensor(
                        out=tmp[:, :w, :], in0=y4[:, :w, :],
                        in1=awrep[:, None, :].to_broadcast([P, w, CH]), op=AL.mult)
                    beta = zpool.tile([P, 4, H], F32, tag="zbeta")
                    nc.vector.tensor_reduce(
                        out=beta[:, :w, :],
                        in_=tmp[:, :w, :].rearrange("p t (h d) -> p t h d", d=DH),
                        axis=mybir.AxisListType.X, op=AL.add)
                    al_ = zpool.tile([P, 4, H], F32, tag="zal")
                    nc.vector.tensor_tensor(
                        out=al_[:, :w, :], in0=beta[:, :w, :],
                        in1=invc[:, t0:t0 + w, None].to_broadcast([P, w, H]),
                        op=AL.mult)
                    al2 = zpool.tile([P, 4, H], F32, tag="zal2")
                    nc.vector.tensor_scalar_mul(al2[:, :w, :], al_[:, :w, :], NEG_SLOPE)
                    sal = zpool.tile([P, 4, H], F32, tag="zsal")
                    nc.vector.tensor_tensor(out=sal[:, :w, :], in0=al_[:, :w, :],
                                            in1=al2[:, :w, :], op=AL.max)
                    zrow = zpool.tile([P, 4, CH + H], BF16, tag="zrow")
                    expS = zrow[:, :w, CH:CH + H]
                    nc.scalar.activation(out=expS, in_=sal[:, :w, :], func=AF.Exp)
                    s4 = zpool.tile([P, 4, H], F32, tag="zs4")
                    nc.vector.tensor_tensor(
                        out=s4[:, :w, :], in0=expS,
                        in1=invc[:, t0:t0 + w, None].to_broadcast([P, w, H]),
                        op=AL.mult)
                    nc.vector.tensor_tensor(
                        out=zrow[:, :w, :CH].rearrange("p t (h d) -> p t h d", d=DH),
                        in0=y4[:, :w, :].rearrange("p t (h d) -> p t h d", d=DH),
                        in1=s4[:, :w, :, None].to_broadcast([P, w, H, DH]),
                        op=AL.mult)
                    nc.sync.dma_start(
                        z_table[t0 * P:(t0 + w) * P, :CH + H].rearrange(
                            "(t p) f -> p t f", p=P),
                        zrow[:, :w, :])
                if debug_tables:
                    nc.sync.dma_start(dbg["z"][:], z_table[:])

            # ================= Phase 5: e2v + ELU + LN =================
            with tc.tile_pool(name="e2vsb", bufs=3) as gpool2, \
                 tc.tile_pool(name="e2vsel", bufs=3) as selpool2, \
                 tc.tile_pool(name="e2vev", bufs=2) as evpool2, \
                 tc.tile_pool(name="e2vidx", bufs=1) as ipool2, \
                 tc.tile_pool(name="e2vps", bufs=3, space="PSUM") as e2vps:
                e2vi = ipool2.tile([P, e2v_tot // 16], I16)
                nc.sync.dma_start(e2vi[:], e2vi_in[:])
                e2vr = ipool2.tile([P, e2v_tot // P], BF16)
                nc.sync.dma_start(e2vr[:], e2vr_in[:])

                call_sizes = _n_gather_calls(sum(e2v_chunks), GATHER_CALL_E2V)
                gtiles = [None] * len(call_sizes)
                tot = sum(e2v_chunks)
                sel_cur, sel0 = None, 0
                elu4, l0, lw = None, 0, 0
                mm = 0
                for g in range(VG):
                    psf = e2vps.tile([P, 512], F32, tag="e2vps")
                    ps = psf[:, :CH + H]
                    for k in range(e2v_chunks[g]):
                        gc, j = divmod(mm, GATHER_CALL_E2V // P)
                        if gtiles[gc] is None:
                            n = call_sizes[gc]
                            gt = gpool2.tile([P, GATHER_CALL_E2V // P, ZW], BF16,
                                             tag="e2vgather")
                            s = gc * GATHER_CALL_E2V
                            nc.gpsimd.dma_gather(
                                gt[:, :n // P, :], z_table[:],
                                e2vi[:, s // 16:(s + n) // 16], n, n, ZW,
                                single_packet=False, queue_num=gc % 2)
                            gtiles[gc] = gt
                        if mm % SELW == 0:
                            sel0 = mm
                            sw = min(SELW, tot - mm)
                            sel_cur = selpool2.tile([P, SELW, P], BF16, tag="e2vsel")
                            nc.vector.tensor_tensor(
                                out=sel_cur[:, :sw, :],
                                in0=e2vr[:, mm:mm + sw, None].to_broadcast([P, sw, P]),
                                in1=iota[:, None, :].to_broadcast([P, sw, P]),
                                op=AL.is_equal)
                        nc.tensor.matmul(ps, lhsT=sel_cur[:, mm - sel0, :],
                                         rhs=gtiles[gc][:, j, :CH + H],
                                         start=(k == 0), stop=(k == e2v_chunks[g] - 1))
                        mm += 1
                    # xpre = num/den ; ELU = exp(min(x,0)) - 1 + relu(x)
                    den = evpool2.tile([P, H], F32, tag="den")
                    nc.vector.tensor_scalar_max(den[:], ps[:, CH:CH + H], 1e-12)
                    rec = evpool2.tile([P, H], F32, tag="rec")
                    nc.vector.reciprocal(rec[:], den[:])
                    xpre = evpool2.tile([P, CH], BF16, tag="xpre")
                    nc.vector.tensor_tensor(
                        out=xpre[:].rearrange("p (h d) -> p h d", d=DH),
                        in0=ps[:, :CH].rearrange("p (h d) -> p h d", d=DH),
                        in1=rec[:, :, None].to_broadcast([P, H, DH]),
                        op=AL.mult)
                    relx = evpool2.tile([P, CH], BF16, tag="relx")
                    nc.scalar.activation(out=relx[:], in_=xpre[:], func=AF.Relu)
                    m0 = evpool2.tile([P, CH], BF16, tag="m0")
                    nc.vector.tensor_tensor(out=m0[:], in0=xpre[:], in1=relx[:],
                                            op=AL.subtract)
                    ep = evpool2.tile([P, CH], F32, tag="ep")
                    nc.scalar.activation(out=ep[:], in_=m0[:], func=AF.Exp)
                    if g % 4 == 0:
                        l0 = g
                        lw = min(4, VG - l0)
                        elu4 = evpool2.tile([P, 4, CH], BF16, tag="elu4")
                    nc.vector.scalar_tensor_tensor(
                        out=elu4[:, g - l0, :], in0=ep[:], scalar=-1.0, in1=relx[:],
                        op0=AL.add, op1=AL.add)
                    if g - l0 == lw - 1:
                        mu4 = evpool2.tile([P, 4], F32, tag="mu4")
                        nc.vector.tensor_reduce(out=mu4[:, :lw], in_=elu4[:, :lw, :],
                                                axis=mybir.AxisListType.X, op=AL.add)
                        nc.vector.tensor_scalar_mul(mu4[:, :lw], mu4[:, :lw], 1.0 / CH)
                        xc4 = evpool2.tile([P, 4, CH], BF16, tag="xc4")
                        nc.vector.tensor_tensor(
                            out=xc4[:, :lw, :], in0=elu4[:, :lw, :],
                            in1=mu4[:, :lw, None].to_broadcast([P, lw, CH]),
                            op=AL.subtract)
                        sq4 = evpool2.tile([P, 4, CH], BF16, tag="sq4")
                        nc.vector.tensor_tensor(out=sq4[:, :lw, :], in0=xc4[:, :lw, :],
                                                in1=xc4[:, :lw, :], op=AL.mult)
                        ss4 = evpool2.tile([P, 4], F32, tag="ss4")
                        nc.vector.tensor_reduce(out=ss4[:, :lw], in_=sq4[:, :lw, :],
                                                axis=mybir.AxisListType.X, op=AL.add)
                        var4 = evpool2.tile([P, 4], F32, tag="var4")
                        nc.vector.tensor_scalar(var4[:, :lw], ss4[:, :lw], 1.0 / CH,
                                                LN_EPS, AL.mult, AL.add)
                        lnv4 = evpool2.tile([P, 4], F32, tag="lnv4")
                        nc.scalar.activation(out=lnv4[:, :lw], in_=var4[:, :lw],
                                             func=AF.Ln)
                        rstd4 = evpool2.tile([P, 4], F32, tag="rstd4")
                        nc.scalar.activation(out=rstd4[:, :lw], in_=lnv4[:, :lw],
                                             func=AF.Exp, scale=-0.5)
                        t4 = evpool2.tile([P, 4, CH], BF16, tag="t4")
                        nc.vector.tensor_tensor(
                            out=t4[:, :lw, :], in0=xc4[:, :lw, :],
                            in1=rstd4[:, :lw, None].to_broadcast([P, lw, CH]),
                            op=AL.mult)
                        t5 = evpool2.tile([P, 4, CH], F32, tag="t5")
                        nc.vector.tensor_tensor(
                            out=t5[:, :lw, :], in0=t4[:, :lw, :],
                            in1=lnwrep[:, None, :].to_broadcast([P, lw, CH]),
                            op=AL.mult)
                        nc.vector.tensor_tensor(
                            out=xn_sb[:, l0:l0 + lw, :], in0=t5[:, :lw, :],
                            in1=lnbrep[:, None, :].to_broadcast([P, lw, CH]),
                            op=AL.add)

            if debug_tables:
                with tc.tile_pool(name="dbgxn", bufs=1) as dxp:
                    dx = dxp.tile([P, VG, CH], BF16)
                    nc.vector.tensor_copy(out=dx[:], in_=xn_sb[:])
                    nc.sync.dma_start(dbg["xn"][:].rearrange("(t p) f -> p t f", p=P),
                                      dx[:])

            tc.strict_bb_all_engine_barrier()

            # ================= Phase 6: GELU + conv + residual =============
            with tc.tile_pool(name="fsb", bufs=3) as fpool, \
                 tc.tile_pool(name="fps", bufs=2, space="PSUM") as fps, \
                 tc.tile_pool(name="ftps", bufs=4, space="PSUM") as ftps:
                for g0 in range(0, VG, 4):
                    w4 = min(4, VG - g0)
                    xg4 = fpool.tile([P, 4, CH], BF16, tag="xg4")
                    nc.scalar.activation(out=xg4[:, :w4, :],
                                         in_=xn_sb[:, g0:g0 + w4, :], func=AF.Gelu)
                    x4 = fpool.tile([P, 4, CH], F32, tag="x4")
                    nc.sync.dma_start(x4[:, :w4, :], rows(x_in, g0, w4))
                    xgc4 = fpool.tile([P, 4, CH], F32, tag="xgc4")
                    nc.vector.tensor_tensor(
                        out=xgc4[:, :w4, :], in0=x4[:, :w4, :],
                        in1=gcb[:, None, :].to_broadcast([P, w4, CH]), op=AL.add)
                    ofin4 = fpool.tile([P, 4, CH], F32, tag="ofin4")
                    for j in range(w4):
                        xgT = fpool.tile([P, 2, P], BF16, tag="xgT")
                        for k in range(2):
                            tp = ftps.tile([P, P], BF16, tag="tps")
                            nc.tensor.transpose(tp[:], xg4[:, j, k * P:(k + 1) * P],
                                                ident[:])
                            nc.scalar.copy(out=xgT[:, k, :], in_=tp[:])
                        psf = fps.tile([P, 512], F32, tag="fps")
                        ps = psf[:, :CH]
                        for k in range(2):
                            nc.tensor.matmul(ps, lhsT=xgT[:, k, :],
                                             rhs=convw_sb[:, k, :],
                                             start=(k == 0), stop=(k == 1))
                        xo = fpool.tile([P, CH], F32, tag="xo")
                        nc.vector.tensor_tensor(out=xo[:], in0=ps, in1=gammarep[:],
                                                op=AL.mult)
                        nc.vector.tensor_tensor(out=ofin4[:, j, :], in0=xo[:],
                                                in1=xgc4[:, j, :], op=AL.add)
                    nc.sync.dma_start(rows(out_ext, g0, w4), ofin4[:, :w4, :])

    nc.compile()
    return nc


def prepare_inputs(X, edge_idx, vertex_idx, theta_w, theta_b, atten_w,
                   ln_w, ln_b, conv_w, conv_b, gamma, plan):
    X = np.asarray(X, dtype=np.float32)
    theta_w = np.asarray(theta_w, dtype=np.float32)
    wcat = _bf(theta_w.transpose(1, 0, 2).reshape(CH, CH))
    brep = np.tile(np.asarray(theta_b, np.float32).reshape(1, CH), (P, 1))
    awrep = _bf(np.tile(np.asarray(atten_w, np.float32).reshape(1, CH), (P, 1)))
    convw = _bf(np.asarray(conv_w, np.float32))
    convbrep = np.tile(np.asarray(conv_b, np.float32).reshape(1, CH), (P, 1))
    gammarep = np.tile(np.asarray(gamma, np.float32).reshape(1, CH), (P, 1))
    lnwrep = np.tile(np.asarray(ln_w, np.float32).reshape(1, CH), (P, 1))
    lnbrep = np.tile(np.asarray(ln_b, np.float32).reshape(1, CH), (P, 1))
    iota = _bf(np.tile(np.arange(P, dtype=np.float32), (P, 1)))
    ident = _bf(np.eye(P, dtype=np.float32))
    invc = np.ascontiguousarray(
        plan["inv_cnt"].reshape(EG, P).T).astype(np.float32)

    in_maps = []
    for c in range(NCORES):
        xc = np.zeros((NPC_PAD, CH), np.float32)
        xc[:NPC] = X[c * NPC:(c + 1) * NPC]
        xcb = _bf(xc)
        xt = np.ascontiguousarray(
            xcb.reshape(VG, P, 2, P).transpose(3, 0, 2, 1)).reshape(P, VG * 2 * P)
        in_maps.append(dict(
            x=xc, xt=_bf(xt), wcat=wcat, convw=convw,
            brep=brep.astype(np.float32), awrep=awrep,
            convbrep=convbrep.astype(np.float32),
            gammarep=gammarep.astype(np.float32),
            lnwrep=lnwrep.astype(np.float32), lnbrep=lnbrep.astype(np.float32),
            iota=iota, ident=ident, invc=invc,
            v2ei=_wrap16(plan["v2e_idx"][c]),
            v2er=_pairmajor(plan["v2e_rel"][c], ml_dtypes.bfloat16),
            e2vi=_wrap16(plan["e2v_idx"][c]),
            e2vr=_pairmajor(plan["e2v_rel"][c], ml_dtypes.bfloat16),
        ))
    return in_maps


_CACHE = {}


def kernel(X, edge_idx, vertex_idx, theta_w, theta_b, atten_w,
           ln_w, ln_b, conv_w, conv_b, gamma):
    debug_tables = bool(int(os.environ.get("GNN_DEBUG_TABLES", "0")))
    trace = bool(int(os.environ.get("GNN_TRACE", "0")))

    plan = make_plan(edge_idx, vertex_idx)
    key = (tuple(plan["v2e_chunks"]), tuple(plan["e2v_chunks"]), debug_tables)
    if key not in _CACHE:
        _CACHE[key] = build_kernel(plan["v2e_chunks"], plan["e2v_chunks"],
                                   debug_tables=debug_tables)
    nc = _CACHE[key]

    in_maps = prepare_inputs(X, edge_idx, vertex_idx, theta_w, theta_b,
                             atten_w, ln_w, ln_b, conv_w, conv_b, gamma, plan)
    res = run_bass_kernel_spmd(nc, in_maps, core_ids=list(range(NCORES)),
                               trace=trace)
    kernel.last_results = res
    out = np.concatenate(
        [np.asarray(res.results[c]["out"])[:NPC] for c in range(NCORES)], axis=0)
    return out.astype(np.float32)


# revision 13
# speedup vs baseline: 1.9582x; 1.2860x over previous
"""Trainium2 8-core Bass kernel for the UniGAT hypergraph attention block.

Algorithm (matches the jax reference numerically, up to bf16 rounding):
  1. Xh = X @ theta_cat + b          (per-core node shard, PE matmul)
  2. v2e: esum[e] = sum over incidence pairs (e,v) of Xh[v]
       - per-core partial over its node shard: dma_gather of Xh rows per
         pair (sorted by edge) + 0/1-indicator segment matmul on PE
       - AllReduce(esum) over the 8 cores
  3. Softmax folding: w = exp(s)/sum(exp(s)) exactly (the segment-max
     subtraction cancels; s = leaky_relu in [-0.5, 0.5] so exp is safe).
     Build per-edge table Z = [Y*expS | expS] where Y = esum*inv_cnt,
     expS[e,h] = exp(leaky_relu(inv_cnt*(esum @ aw_h))).
  4. e2v: plain 0/1 segment-sum of gathered Z rows per destination vertex
     (sorted by vertex) -> numerator (256 cols) and denominator (4 cols);
     divide per head.
  5. ELU -> LayerNorm -> GELU -> conv matmul -> X + gamma * Xo.

Sharding: nodes (and pairs grouped by destination vertex) across 8 cores;
weights and edge tables replicated; one AllReduce of esum is the only
collective.
"""

import os

import numpy as np
import ml_dtypes

import concourse.bass as bass
import concourse.bacc as bacc
import concourse.tile as tile
import concourse.mybir as mybir
from concourse.bass_utils import run_bass_kernel_spmd
from concourse.library_config import mlp

BF16 = mybir.dt.bfloat16
F32 = mybir.dt.float32
I16 = mybir.dt.int16
AL = mybir.AluOpType
AF = mybir.ActivationFunctionType

P = 128
NCORES = 8

N_NODES = 100000
N_EDGES = 20000
NNZ = 500000
CH = 256
H = 4
DH = 64
NEG_SLOPE = 0.2
LN_EPS = 1e-6

NPC = N_NODES // NCORES          # 12500
VG = (NPC + P - 1) // P          # 98
NPC_PAD = VG * P                 # 12544
EG = (N_EDGES + P - 1) // P      # 157
E_PAD = EG * P                   # 20096

ZW = 384                         # Z table row stride (260 used; %128 elems)
GATHER_CALL_V2E = 4096
GATHER_CALL_E2V = 4096
SELW = 8                         # indicator chunks built per DVE op


def _bf(x):
    return np.asarray(x, dtype=ml_dtypes.bfloat16)


def _wrap16(idx):
    """dma_gather index layout: index i -> [i % 16, i // 16], replicated x8."""
    assert idx.size % 16 == 0
    w = idx.reshape(-1, 16).T
    return np.ascontiguousarray(np.tile(w, (8, 1))).astype(np.int16)


def _pairmajor(vals, dtype):
    """pair i -> [i % 128, i // 128]."""
    assert vals.size % P == 0
    return np.ascontiguousarray(vals.reshape(-1, P).T).astype(dtype)


def make_plan(edge_idx, vertex_idx):
    """Host-side index preprocessing (graph structure only)."""
    edge_idx = np.asarray(edge_idx).astype(np.int64)
    vertex_idx = np.asarray(vertex_idx).astype(np.int64)
    core = vertex_idx // NPC
    lv = vertex_idx - core * NPC

    v2e_e, v2e_lv = [], []
    e2v_e, e2v_lv = [], []
    for c in range(NCORES):
        m = core == c
        e_c, lv_c = edge_idx[m], lv[m]
        o = np.argsort(e_c, kind="stable")
        v2e_e.append(e_c[o])
        v2e_lv.append(lv_c[o])
        o = np.argsort(lv_c, kind="stable")
        e2v_e.append(e_c[o])
        e2v_lv.append(lv_c[o])

    def group_counts(keys_list, ngroups):
        cnts = np.zeros((NCORES, ngroups), dtype=np.int64)
        for c in range(NCORES):
            cnts[c] = np.bincount(keys_list[c] // P, minlength=ngroups)
        chunks = np.maximum(1, -(-cnts.max(axis=0) // P))
        return cnts, chunks

    v2e_cnts, v2e_chunks = group_counts(v2e_e, EG)
    e2v_cnts, e2v_chunks = group_counts(e2v_lv, VG)

    def build_streams(vals, cnts, chunks, ngroups, relmod_keys):
        T = int(chunks.sum()) * P
        idx_s = np.zeros((NCORES, T), dtype=np.int64)
        rel_s = np.full((NCORES, T), 255.0, dtype=np.float32)
        starts = np.concatenate([[0], np.cumsum(chunks)]) * P
        for c in range(NCORES):
            gstart = np.concatenate([[0], np.cumsum(cnts[c])])
            for g in range(ngroups):
                n = cnts[c][g]
                if n == 0:
                    continue
                s, d = gstart[g], starts[g]
                idx_s[c, d:d + n] = vals[c][s:s + n]
                rel_s[c, d:d + n] = relmod_keys[c][s:s + n] % P
        return idx_s, rel_s

    v2e_idx, v2e_rel = build_streams(v2e_lv, v2e_cnts, v2e_chunks, EG, v2e_e)
    e2v_idx, e2v_rel = build_streams(e2v_e, e2v_cnts, e2v_chunks, VG, e2v_lv)

    cnt = np.bincount(edge_idx, minlength=E_PAD).astype(np.float32)
    inv_cnt = 1.0 / np.maximum(cnt, 1.0)

    return dict(
        v2e_chunks=[int(x) for x in v2e_chunks],
        e2v_chunks=[int(x) for x in e2v_chunks],
        v2e_idx=v2e_idx, v2e_rel=v2e_rel,
        e2v_idx=e2v_idx, e2v_rel=e2v_rel,
        inv_cnt=inv_cnt,
    )


def _n_gather_calls(total_chunks, call_pairs):
    total = total_chunks * P
    n_full, rem = divmod(total, call_pairs)
    sizes = [call_pairs] * n_full
    if rem:
        sizes.append(rem)
    return sizes


def build_kernel(v2e_chunks, e2v_chunks, debug_tables=False):
    v2e_tot = sum(v2e_chunks) * P
    e2v_tot = sum(e2v_chunks) * P

    nc = bacc.Bacc("TRN2", target_bir_lowering=False, debug=False,
                   num_devices=NCORES, num_swdge_queues=2,
                   dynamic_dma_scratch_size=32768)

    x_in = nc.dram_tensor("x", [NPC_PAD, CH], F32, kind="ExternalInput")
    xt_in = nc.dram_tensor("xt", [P, VG * 2 * P], BF16, kind="ExternalInput")
    wcat_in = nc.dram_tensor("wcat", [CH, CH], BF16, kind="ExternalInput")
    convw_in = nc.dram_tensor("convw", [CH, CH], BF16, kind="ExternalInput")
    brep_in = nc.dram_tensor("brep", [P, CH], F32, kind="ExternalInput")
    awrep_in = nc.dram_tensor("awrep", [P, CH], BF16, kind="ExternalInput")
    convbrep_in = nc.dram_tensor("convbrep", [P, CH], F32, kind="ExternalInput")
    gammarep_in = nc.dram_tensor("gammarep", [P, CH], F32, kind="ExternalInput")
    lnwrep_in = nc.dram_tensor("lnwrep", [P, CH], F32, kind="ExternalInput")
    lnbrep_in = nc.dram_tensor("lnbrep", [P, CH], F32, kind="ExternalInput")
    iota_in = nc.dram_tensor("iota", [P, P], BF16, kind="ExternalInput")
    ident_in = nc.dram_tensor("ident", [P, P], BF16, kind="ExternalInput")
    invc_in = nc.dram_tensor("invc", [P, EG], F32, kind="ExternalInput")
    c14_in = nc.dram_tensor("c14", [P, H], F32, kind="ExternalInput")
    epscol_in = nc.dram_tensor("epscol", [P, 1], F32, kind="ExternalInput")
    v2ei_in = nc.dram_tensor("v2ei", [P, v2e_tot // 16], I16, kind="ExternalInput")
    v2er_in = nc.dram_tensor("v2er", [P, v2e_tot // P], BF16, kind="ExternalInput")
    e2vi_in = nc.dram_tensor("e2vi", [P, e2v_tot // 16], I16, kind="ExternalInput")
    e2vr_in = nc.dram_tensor("e2vr", [P, e2v_tot // P], BF16, kind="ExternalInput")
    out_ext = nc.dram_tensor("out", [NPC_PAD, CH], F32, kind="ExternalOutput")
    dbg = {}
    if debug_tables:
        dbg["xh"] = nc.dram_tensor("dbg_xh", [NPC_PAD, CH], BF16, kind="ExternalOutput")
        dbg["esum"] = nc.dram_tensor("dbg_esum", [E_PAD, CH], BF16, kind="ExternalOutput")
        dbg["z"] = nc.dram_tensor("dbg_z", [E_PAD, ZW], BF16, kind="ExternalOutput")
        dbg["xn"] = nc.dram_tensor("dbg_xn", [NPC_PAD, CH], BF16, kind="ExternalOutput")

    def rows(dr, t0, w):
        return dr[t0 * P:(t0 + w) * P, :].rearrange("(t p) f -> p t f", p=P)

    with tile.TileContext(nc) as tc:
        with tc.tile_pool(name="dram", bufs=1, space="DRAM") as dram, \
             tc.tile_pool(name="const", bufs=1) as cpool, \
             tc.tile_pool(name="resident", bufs=1) as rpool:

            nc.gpsimd.load_library(mlp)

            xh_table = dram.tile([NPC_PAD, CH], BF16)
            esum_bounce = dram.tile([E_PAD, CH], BF16)
            AR_BOUNDS = [40, 80, 120, EG]
            _ar_lims = list(zip([0] + AR_BOUNDS[:-1], AR_BOUNDS))
            yfulls = []
            for _ci, (_a, _b) in enumerate(_ar_lims):
                yf = dram.tile([(_b - _a) * P, CH], BF16, addr_space="Shared",
                               name=f"yfull{_ci}", tag=f"yfull{_ci}")
                yfulls.append(yf)
            z_table = dram.tile([E_PAD, ZW], BF16)

            def yrows(t0, w):
                """rows [t0*128,(t0+w)*128) of the chunked AR output; the
                caller must not cross an AR chunk boundary."""
                for (_a, _b), yf in zip(_ar_lims, yfulls):
                    if t0 >= _a and t0 + w <= _b:
                        return yf[(t0 - _a) * P:(t0 - _a + w) * P, :].rearrange(
                            "(t p) f -> p t f", p=P)
                raise AssertionError("yrows crosses AR chunk")

            def cload(dr, shape, dtype, name):
                t = cpool.tile(shape, dtype, name=name, tag=name)
                nc.sync.dma_start(t[:], dr[:])
                return t

            w_sb = cpool.tile([P, 2, CH], BF16)
            nc.sync.dma_start(w_sb[:], wcat_in[:].rearrange("(k p) f -> p k f", p=P))
            convw_sb = cpool.tile([P, 2, CH], BF16)
            nc.sync.dma_start(convw_sb[:], convw_in[:].rearrange("(k p) f -> p k f", p=P))
            brep = cload(brep_in, [P, CH], F32, "brep")
            awrep = cload(awrep_in, [P, CH], BF16, "awrep")
            convbrep = cload(convbrep_in, [P, CH], F32, "convbrep")
            gammarep = cload(gammarep_in, [P, CH], F32, "gammarep")
            lnwrep = cload(lnwrep_in, [P, CH], F32, "lnwrep")
            lnbrep = cload(lnbrep_in, [P, CH], F32, "lnbrep")
            iota = cload(iota_in, [P, P], BF16, "iota")
            ident = cload(ident_in, [P, P], BF16, "ident")
            invc = cload(invc_in, [P, EG], F32, "invc")
            c14 = cload(c14_in, [P, H], F32, "c14")
            epscol = cload(epscol_in, [P, 1], F32, "epscol")
            gcb = cpool.tile([P, CH], F32)
            nc.vector.tensor_tensor(out=gcb[:], in0=gammarep[:], in1=convbrep[:],
                                    op=AL.mult)

            xn_sb = rpool.tile([P, VG, CH], BF16)

            # ================= Phase 1: Xh = X @ W + b =================
            with tc.tile_pool(name="p1sb", bufs=3) as p1sb, \
                 tc.tile_pool(name="p1xt", bufs=1) as p1xt, \
                 tc.tile_pool(name="p1ps", bufs=2, space="PSUM") as p1ps:
                xt_sb = p1xt.tile([P, VG * 2 * P], BF16, tag="xt")
                nc.sync.dma_start(xt_sb[:], xt_in[:])
                xt_v = xt_sb[:].rearrange("p (t k f) -> p t k f", t=VG, k=2)
                xh4, t0, tw = None, 0, 0
                for t in range(VG):
                    psf = p1ps.tile([P, 512], F32, tag="xhps")
                    ps = psf[:, :CH]
                    for k in range(2):
                        nc.tensor.matmul(ps, lhsT=xt_v[:, t, k, :], rhs=w_sb[:, k, :],
                                         start=(k == 0), stop=(k == 1))
                    if t % 4 == 0:
                        t0 = t
                        tw = min(4, VG - t0)
                        xh4 = p1sb.tile([P, 4, CH], BF16, tag="xhout")
                    nc.vector.tensor_tensor(out=xh4[:, t - t0, :], in0=ps,
                                            in1=brep[:], op=AL.add)
                    if t - t0 == tw - 1:
                        nc.sync.dma_start(rows(xh_table, t0, tw), xh4[:, :tw, :])
                if debug_tables:
                    nc.sync.dma_start(dbg["xh"][:], xh_table[:])

            # ================= Phase 2: v2e partial esum =================
            with tc.tile_pool(name="v2esb", bufs=4) as gpool, \
                 tc.tile_pool(name="v2esel", bufs=3) as selpool, \
                 tc.tile_pool(name="v2eev", bufs=3) as evpool, \
                 tc.tile_pool(name="v2eidx", bufs=1) as ipool, \
                 tc.tile_pool(name="v2eps", bufs=3, space="PSUM") as v2eps:
                v2ei = ipool.tile([P, v2e_tot // 16], I16)
                nc.sync.dma_start(v2ei[:], v2ei_in[:])
                v2er = ipool.tile([P, v2e_tot // P], BF16)
                nc.sync.dma_start(v2er[:], v2er_in[:])

                call_sizes = _n_gather_calls(sum(v2e_chunks), GATHER_CALL_V2E)
                gtiles = [None] * len(call_sizes)
                tot = sum(v2e_chunks)
                sel_cur, sel0 = None, 0
                esb4, e0, ew = None, 0, 0
                mm = 0
                for g in range(EG):
                    psf = v2eps.tile([P, 512], F32, tag="v2eps")
                    ps = psf[:, :CH]
                    for k in range(v2e_chunks[g]):
                        gc, j = divmod(mm, GATHER_CALL_V2E // P)
                        if gtiles[gc] is None:
                            n = call_sizes[gc]
                            gt = gpool.tile([P, GATHER_CALL_V2E // P, CH], BF16,
                                            tag="v2egather")
                            s = gc * GATHER_CALL_V2E
                            nc.gpsimd.dma_gather(
                                gt[:, :n // P, :], xh_table[:],
                                v2ei[:, s // 16:(s + n) // 16], n, n, CH,
                                single_packet=False, queue_num=gc % 2)
                            gtiles[gc] = gt
                        if mm % SELW == 0:
                            sel0 = mm
                            sw = min(SELW, tot - mm)
                            sel_cur = selpool.tile([P, SELW, P], BF16, tag="v2esel")
                            nc.vector.tensor_tensor(
                                out=sel_cur[:, :sw, :],
                                in0=v2er[:, mm:mm + sw, None].to_broadcast([P, sw, P]),
                                in1=iota[:, None, :].to_broadcast([P, sw, P]),
                                op=AL.is_equal)
                        nc.tensor.matmul(ps, lhsT=sel_cur[:, mm - sel0, :],
                                         rhs=gtiles[gc][:, j, :],
                                         start=(k == 0), stop=(k == v2e_chunks[g] - 1))
                        mm += 1
                    if g % 4 == 0:
                        e0 = g
                        ew = min(4, EG - e0)
                        esb4 = evpool.tile([P, 4, CH], BF16, tag="v2eev")
                    nc.vector.tensor_copy(out=esb4[:, g - e0, :], in_=ps)
                    if g - e0 == ew - 1:
                        nc.sync.dma_start(rows(esum_bounce, e0, ew), esb4[:, :ew, :])
                    if g + 1 in AR_BOUNDS:
                        ci = AR_BOUNDS.index(g + 1)
                        a = 0 if ci == 0 else AR_BOUNDS[ci - 1]
                        nc.gpsimd.collective_compute(
                            "AllReduce", AL.add,
                            replica_groups=[list(range(NCORES))],
                            ins=[esum_bounce[a * P:(g + 1) * P, :].opt()],
                            outs=[yfulls[ci].opt()])

            # ================= Phase 3: AllReduce (issued chunked in phase 2)
            if debug_tables:
                for (_a, _b), yf in zip(_ar_lims, yfulls):
                    nc.sync.dma_start(dbg["esum"][_a * P:_b * P, :], yf[:])

            # ================= Phase 4: Z table (4 edge tiles per iter) ====
            with tc.tile_pool(name="zsb", bufs=3) as zpool:
                for t0 in range(0, EG, 4):
                    w = min(4, EG - t0)
                    y4 = zpool.tile([P, 4, CH], BF16, tag="zy")
                    nc.sync.dma_start(y4[:, :w, :], yrows(t0, w))
                    tmp = zpool.tile([P, 4, CH], BF16, tag="ztmp")
                    nc.vector.tensor_tensor(
                        out=tmp[:, :w, :], in0=y4[:, :w, :],
                        in1=awrep[:, None, :].to_broadcast([P, w, CH]), op=AL.mult)
                    beta = zpool.tile([P, 4, H], F32, tag="zbeta")
                    nc.vector.tensor_reduce(
                        out=beta[:, :w, :],
                        in_=tmp[:, :w, :].rearrange("p t (h d) -> p t h d", d=DH),
                        axis=mybir.AxisListType.X, op=AL.add)
                    al_ = zpool.tile([P, 4, H], F32, tag="zal")
                    nc.vector.tensor_tensor(
                        out=al_[:, :w, :], in0=beta[:, :w, :],
                        in1=invc[:, t0:t0 + w, None].to_broadcast([P, w, H]),
                        op=AL.mult)
                    sal = zpool.tile([P, 4, H], F32, tag="zsal")
                    nc.scalar.activation(out=sal[:, :w, :], in_=al_[:, :w, :],
                                         func=AF.Prelu, alpha=NEG_SLOPE)
                    zrow = zpool.tile([P, 4, CH + H], BF16, tag="zrow")
                    expS = zrow[:, :w, CH:CH + H]
                    nc.scalar.activation(out=expS, in_=sal[:, :w, :], func=AF.Exp)
                    s4 = zpool.tile([P, 4, H], F32, tag="zs4")
                    nc.vector.tensor_tensor(
                        out=s4[:, :w, :], in0=expS,
                        in1=invc[:, t0:t0 + w, None].to_broadcast([P, w, H]),
                        op=AL.mult)
                    nc.vector.tensor_tensor(
                        out=zrow[:, :w, :CH].rearrange("p t (h d) -> p t h d", d=DH),
                        in0=y4[:, :w, :].rearrange("p t (h d) -> p t h d", d=DH),
                        in1=s4[:, :w, :, None].to_broadcast([P, w, H, DH]),
                        op=AL.mult)
                    nc.sync.dma_start(
                        z_table[t0 * P:(t0 + w) * P, :CH + H].rearrange(
                            "(t p) f -> p t f", p=P),
                        zrow[:, :w, :])
                if debug_tables:
                    nc.sync.dma_start(dbg["z"][:], z_table[:])

            # ================= Phase 5: e2v + ELU + LN =================
            with tc.tile_pool(name="e2vsb", bufs=3) as gpool2, \
                 tc.tile_pool(name="e2vsel", bufs=3) as selpool2, \
                 tc.tile_pool(name="e2vev", bufs=2) as evpool2, \
                 tc.tile_pool(name="e2vidx", bufs=1) as ipool2, \
                 tc.tile_pool(name="e2vps", bufs=3, space="PSUM") as e2vps:
                e2vi = ipool2.tile([P, e2v_tot // 16], I16)
                nc.sync.dma_start(e2vi[:], e2vi_in[:])
                e2vr = ipool2.tile([P, e2v_tot // P], BF16)
                nc.sync.dma_start(e2vr[:], e2vr_in[:])

                call_sizes = _n_gather_calls(sum(e2v_chunks), GATHER_CALL_E2V)
                gtiles = [None] * len(call_sizes)
                tot = sum(e2v_chunks)
                sel_cur, sel0 = None, 0
                elu4, l0, lw = None, 0, 0
                mm = 0
                for g in range(VG):
                    psf = e2vps.tile([P, 512], F32, tag="e2vps")
                    ps = psf[:, :CH + H]
                    for k in range(e2v_chunks[g]):
                        gc, j = divmod(mm, GATHER_CALL_E2V // P)
                        if gtiles[gc] is None:
                            n = call_sizes[gc]
                            gt = gpool2.tile([P, GATHER_CALL_E2V // P, ZW], BF16,
                                             tag="e2vgather")
                            s = gc * GATHER_CALL_E2V
                            nc.gpsimd.dma_gather(
                                gt[:, :n // P, :], z_table[:],
                                e2vi[:, s // 16:(s + n) // 16], n, n, ZW,
                                single_packet=False, queue_num=gc % 2)
                            gtiles[gc] = gt
                        if mm % SELW == 0:
                            sel0 = mm
                            sw = min(SELW, tot - mm)
                            sel_cur = selpool2.tile([P, SELW, P], BF16, tag="e2vsel")
                            nc.vector.tensor_tensor(
                                out=sel_cur[:, :sw, :],
                                in0=e2vr[:, mm:mm + sw, None].to_broadcast([P, sw, P]),
                                in1=iota[:, None, :].to_broadcast([P, sw, P]),
                                op=AL.is_equal)
                        nc.tensor.matmul(ps, lhsT=sel_cur[:, mm - sel0, :],
                                         rhs=gtiles[gc][:, j, :CH + H],
                                         start=(k == 0), stop=(k == e2v_chunks[g] - 1))
                        mm += 1
                    # xpre = num/den ; ELU = exp(min(x,0)) - 1 + relu(x)
                    den = evpool2.tile([P, H], F32, tag="den")
                    nc.vector.tensor_scalar_max(den[:], ps[:, CH:CH + H], 1e-12)
                    rec = evpool2.tile([P, H], F32, tag="rec")
                    nc.vector.reciprocal(rec[:], den[:])
                    xpre = evpool2.tile([P, CH], BF16, tag="xpre")
                    nc.vector.tensor_tensor(
                        out=xpre[:].rearrange("p (h d) -> p h d", d=DH),
                        in0=ps[:, :CH].rearrange("p (h d) -> p h d", d=DH),
                        in1=rec[:, :, None].to_broadcast([P, H, DH]),
                        op=AL.mult)
                    relx = evpool2.tile([P, CH], BF16, tag="relx")
                    nc.scalar.activation(out=relx[:], in_=xpre[:], func=AF.Relu)
                    m0 = evpool2.tile([P, CH], BF16, tag="m0")
                    nc.scalar.activation(out=m0[:], in_=xpre[:], func=AF.Relu,
                                         scale=-1.0)
                    ep = evpool2.tile([P, CH], F32, tag="ep")
                    nc.scalar.activation(out=ep[:], in_=m0[:], func=AF.Exp,
                                         scale=-1.0)
                    if g % 4 == 0:
                        l0 = g
                        lw = min(4, VG - l0)
                        elu4 = evpool2.tile([P, 4, CH], BF16, tag="elu4")
                    nc.vector.scalar_tensor_tensor(
                        out=elu4[:, g - l0, :], in0=ep[:], scalar=-1.0, in1=relx[:],
                        op0=AL.add, op1=AL.add)
                    if g - l0 == lw - 1:
                        mu4 = evpool2.tile([P, 4], F32, tag="mu4")
                        nc.vector.tensor_reduce(out=mu4[:, :lw], in_=elu4[:, :lw, :],
                                                axis=mybir.AxisListType.X, op=AL.add)
                        nc.vector.tensor_tensor(out=mu4[:, :lw], in0=mu4[:, :lw],
                                                in1=c14[:, :lw], op=AL.mult)
                        xc4 = evpool2.tile([P, 4, CH], BF16, tag="xc4")
                        nc.vector.tensor_tensor(
                            out=xc4[:, :lw, :], in0=elu4[:, :lw, :],
                            in1=mu4[:, :lw, None].to_broadcast([P, lw, CH]),
                            op=AL.subtract)
                        sq4 = evpool2.tile([P, 4, CH], BF16, tag="sq4")
                        nc.vector.tensor_tensor(out=sq4[:, :lw, :], in0=xc4[:, :lw, :],
                                                in1=xc4[:, :lw, :], op=AL.mult)
                        ss4 = evpool2.tile([P, 4], F32, tag="ss4")
                        nc.vector.tensor_reduce(out=ss4[:, :lw], in_=sq4[:, :lw, :],
                                                axis=mybir.AxisListType.X, op=AL.add)
                        var4 = evpool2.tile([P, 4], F32, tag="var4")
                        nc.vector.tensor_tensor(out=var4[:, :lw], in0=ss4[:, :lw],
                                                in1=c14[:, :lw], op=AL.mult)
                        lnv4 = evpool2.tile([P, 4], F32, tag="lnv4")
                        nc.scalar.activation(out=lnv4[:, :lw], in_=var4[:, :lw],
                                             func=AF.Ln, bias=epscol[:])
                        rstd4 = evpool2.tile([P, 4], F32, tag="rstd4")
                        nc.scalar.activation(out=rstd4[:, :lw], in_=lnv4[:, :lw],
                                             func=AF.Exp, scale=-0.5)
                        t4 = evpool2.tile([P, 4, CH], BF16, tag="t4")
                        nc.vector.tensor_tensor(
                            out=t4[:, :lw, :], in0=xc4[:, :lw, :],
                            in1=rstd4[:, :lw, None].to_broadcast([P, lw, CH]),
                            op=AL.mult)
                        t5 = evpool2.tile([P, 4, CH], F32, tag="t5")
                        nc.vector.tensor_tensor(
                            out=t5[:, :lw, :], in0=t4[:, :lw, :],
                            in1=lnwrep[:, None, :].to_broadcast([P, lw, CH]),
                            op=AL.mult)
                        nc.vector.tensor_tensor(
                            out=xn_sb[:, l0:l0 + lw, :], in0=t5[:, :lw, :],
                            in1=lnbrep[:, None, :].to_broadcast([P, lw, CH]),
                            op=AL.add)

            if debug_tables:
                with tc.tile_pool(name="dbgxn", bufs=1) as dxp:
                    dx = dxp.tile([P, VG, CH], BF16)
                    nc.vector.tensor_copy(out=dx[:], in_=xn_sb[:])
                    nc.sync.dma_start(dbg["xn"][:].rearrange("(t p) f -> p t f", p=P),
                                      dx[:])

            tc.strict_bb_all_engine_barrier()

            # ================= Phase 6: GELU + conv + residual =============
            with tc.tile_pool(name="fsb", bufs=3) as fpool, \
                 tc.tile_pool(name="fps", bufs=2, space="PSUM") as fps, \
                 tc.tile_pool(name="ftps", bufs=4, space="PSUM") as ftps:
                for g0 in range(0, VG, 4):
                    w4 = min(4, VG - g0)
                    xg4 = fpool.tile([P, 4, CH], BF16, tag="xg4")
                    nc.scalar.activation(out=xg4[:, :w4, :],
                                         in_=xn_sb[:, g0:g0 + w4, :], func=AF.Gelu)
                    x4 = fpool.tile([P, 4, CH], F32, tag="x4")
                    nc.sync.dma_start(x4[:, :w4, :], rows(x_in, g0, w4))
                    xgc4 = fpool.tile([P, 4, CH], F32, tag="xgc4")
                    nc.vector.tensor_tensor(
                        out=xgc4[:, :w4, :], in0=x4[:, :w4, :],
                        in1=gcb[:, None, :].to_broadcast([P, w4, CH]), op=AL.add)
                    ofin4 = fpool.tile([P, 4, CH], F32, tag="ofin4")
                    for j in range(w4):
                        xgT = fpool.tile([P, 2, P], BF16, tag="xgT")
                        for k in range(2):
                            tp = ftps.tile([P, P], BF16, tag="tps")
                            nc.tensor.transpose(tp[:], xg4[:, j, k * P:(k + 1) * P],
                                                ident[:])
                            nc.scalar.copy(out=xgT[:, k, :], in_=tp[:])
                        psf = fps.tile([P, 512], F32, tag="fps")
                        ps = psf[:, :CH]
                        for k in range(2):
                            nc.tensor.matmul(ps, lhsT=xgT[:, k, :],
                                             rhs=convw_sb[:, k, :],
                                             start=(k == 0), stop=(k == 1))
                        xo = fpool.tile([P, CH], F32, tag="xo")
                        nc.vector.tensor_tensor(out=xo[:], in0=ps, in1=gammarep[:],
                                                op=AL.mult)
                        nc.vector.tensor_tensor(out=ofin4[:, j, :], in0=xo[:],
                                                in1=xgc4[:, j, :], op=AL.add)
                    nc.sync.dma_start(rows(out_ext, g0, w4), ofin4[:, :w4, :])

    nc.compile()
    return nc


def prepare_inputs(X, edge_idx, vertex_idx, theta_w, theta_b, atten_w,
                   ln_w, ln_b, conv_w, conv_b, gamma, plan):
    X = np.asarray(X, dtype=np.float32)
    theta_w = np.asarray(theta_w, dtype=np.float32)
    wcat = _bf(theta_w.transpose(1, 0, 2).reshape(CH, CH))
    brep = np.tile(np.asarray(theta_b, np.float32).reshape(1, CH), (P, 1))
    awrep = _bf(np.tile(np.asarray(atten_w, np.float32).reshape(1, CH), (P, 1)))
    convw = _bf(np.asarray(conv_w, np.float32))
    convbrep = np.tile(np.asarray(conv_b, np.float32).reshape(1, CH), (P, 1))
    gammarep = np.tile(np.asarray(gamma, np.float32).reshape(1, CH), (P, 1))
    lnwrep = np.tile(np.asarray(ln_w, np.float32).reshape(1, CH), (P, 1))
    lnbrep = np.tile(np.asarray(ln_b, np.float32).reshape(1, CH), (P, 1))
    iota = _bf(np.tile(np.arange(P, dtype=np.float32), (P, 1)))
    ident = _bf(np.eye(P, dtype=np.float32))
    invc = np.ascontiguousarray(
        plan["inv_cnt"].reshape(EG, P).T).astype(np.float32)
    c14 = np.full((P, H), 1.0 / CH, np.float32)
    epscol = np.full((P, 1), LN_EPS, np.float32)

    in_maps = []
    for c in range(NCORES):
        xc = np.zeros((NPC_PAD, CH), np.float32)
        xc[:NPC] = X[c * NPC:(c + 1) * NPC]
        xcb = _bf(xc)
        xt = np.ascontiguousarray(
            xcb.reshape(VG, P, 2, P).transpose(3, 0, 2, 1)).reshape(P, VG * 2 * P)
        in_maps.append(dict(
            x=xc, xt=_bf(xt), wcat=wcat, convw=convw,
            brep=brep.astype(np.float32), awrep=awrep,
            convbrep=convbrep.astype(np.float32),
            gammarep=gammarep.astype(np.float32),
            lnwrep=lnwrep.astype(np.float32), lnbrep=lnbrep.astype(np.float32),
            iota=iota, ident=ident, invc=invc, c14=c14, epscol=epscol,
            v2ei=_wrap16(plan["v2e_idx"][c]),
            v2er=_pairmajor(plan["v2e_rel"][c], ml_dtypes.bfloat16),
            e2vi=_wrap16(plan["e2v_idx"][c]),
            e2vr=_pairmajor(plan["e2v_rel"][c], ml_dtypes.bfloat16),
        ))
    return in_maps


_CACHE = {}


def kernel(X, edge_idx, vertex_idx, theta_w, theta_b, atten_w,
           ln_w, ln_b, conv_w, conv_b, gamma):
    debug_tables = bool(int(os.environ.get("GNN_DEBUG_TABLES", "0")))
    trace = bool(int(os.environ.get("GNN_TRACE", "0")))

    plan = make_plan(edge_idx, vertex_idx)
    key = (tuple(plan["v2e_chunks"]), tuple(plan["e2v_chunks"]), debug_tables)
    if key not in _CACHE:
        _CACHE[key] = build_kernel(plan["v2e_chunks"], plan["e2v_chunks"],
                                   debug_tables=debug_tables)
    nc = _CACHE[key]

    in_maps = prepare_inputs(X, edge_idx, vertex_idx, theta_w, theta_b,
                             atten_w, ln_w, ln_b, conv_w, conv_b, gamma, plan)
    res = run_bass_kernel_spmd(nc, in_maps, core_ids=list(range(NCORES)),
                               trace=trace)
    kernel.last_results = res
    out = np.concatenate(
        [np.asarray(res.results[c]["out"])[:NPC] for c in range(NCORES)], axis=0)
    return out.astype(np.float32)


# revision 15
# speedup vs baseline: 1.9614x; 1.0016x over previous
"""Trainium2 8-core Bass kernel for the UniGAT hypergraph attention block.

Algorithm (matches the jax reference numerically, up to bf16 rounding):
  1. Xh = X @ theta_cat + b          (per-core node shard, PE matmul)
  2. v2e: esum[e] = sum over incidence pairs (e,v) of Xh[v]
       - per-core partial over its node shard: dma_gather of Xh rows per
         pair (sorted by edge) + 0/1-indicator segment matmul on PE
       - AllReduce(esum) over the 8 cores
  3. Softmax folding: w = exp(s)/sum(exp(s)) exactly (the segment-max
     subtraction cancels; s = leaky_relu in [-0.5, 0.5] so exp is safe).
     Build per-edge table Z = [Y*expS | expS] where Y = esum*inv_cnt,
     expS[e,h] = exp(leaky_relu(inv_cnt*(esum @ aw_h))).
  4. e2v: plain 0/1 segment-sum of gathered Z rows per destination vertex
     (sorted by vertex) -> numerator (256 cols) and denominator (4 cols);
     divide per head.
  5. ELU -> LayerNorm -> GELU -> conv matmul -> X + gamma * Xo.

Sharding: nodes (and pairs grouped by destination vertex) across 8 cores;
weights and edge tables replicated; one AllReduce of esum is the only
collective.
"""

import os

import numpy as np
import ml_dtypes

import concourse.bass as bass
import concourse.bacc as bacc
import concourse.tile as tile
import concourse.mybir as mybir
from concourse.bass_utils import run_bass_kernel_spmd
from concourse.library_config import mlp

BF16 = mybir.dt.bfloat16
F32 = mybir.dt.float32
I16 = mybir.dt.int16
AL = mybir.AluOpType
AF = mybir.ActivationFunctionType

P = 128
NCORES = 8

N_NODES = 100000
N_EDGES = 20000
NNZ = 500000
CH = 256
H = 4
DH = 64
NEG_SLOPE = 0.2
LN_EPS = 1e-6

NPC = N_NODES // NCORES          # 12500
VG = (NPC + P - 1) // P          # 98
NPC_PAD = VG * P                 # 12544
EG = (N_EDGES + P - 1) // P      # 157
E_PAD = EG * P                   # 20096

ZW = 384                         # Z table row stride (260 used; %128 elems)
GATHER_CALL_V2E = 4096
GATHER_CALL_E2V = 4096
SELW = 16                        # indicator chunks built per DVE op


def _bf(x):
    return np.asarray(x, dtype=ml_dtypes.bfloat16)


def _wrap16(idx):
    """dma_gather index layout: index i -> [i % 16, i // 16], replicated x8."""
    assert idx.size % 16 == 0
    w = idx.reshape(-1, 16).T
    return np.ascontiguousarray(np.tile(w, (8, 1))).astype(np.int16)


def _pairmajor(vals, dtype):
    """pair i -> [i % 128, i // 128]."""
    assert vals.size % P == 0
    return np.ascontiguousarray(vals.reshape(-1, P).T).astype(dtype)


def make_plan(edge_idx, vertex_idx):
    """Host-side index preprocessing (graph structure only)."""
    edge_idx = np.asarray(edge_idx).astype(np.int64)
    vertex_idx = np.asarray(vertex_idx).astype(np.int64)
    core = vertex_idx // NPC
    lv = vertex_idx - core * NPC

    v2e_e, v2e_lv = [], []
    e2v_e, e2v_lv = [], []
    for c in range(NCORES):
        m = core == c
        e_c, lv_c = edge_idx[m], lv[m]
        o = np.argsort(e_c, kind="stable")
        v2e_e.append(e_c[o])
        v2e_lv.append(lv_c[o])
        o = np.argsort(lv_c, kind="stable")
        e2v_e.append(e_c[o])
        e2v_lv.append(lv_c[o])

    def group_counts(keys_list, ngroups):
        cnts = np.zeros((NCORES, ngroups), dtype=np.int64)
        for c in range(NCORES):
            cnts[c] = np.bincount(keys_list[c] // P, minlength=ngroups)
        chunks = np.maximum(1, -(-cnts.max(axis=0) // P))
        return cnts, chunks

    v2e_cnts, v2e_chunks = group_counts(v2e_e, EG)
    e2v_cnts, e2v_chunks = group_counts(e2v_lv, VG)

    def build_streams(vals, cnts, chunks, ngroups, relmod_keys):
        T = int(chunks.sum()) * P
        idx_s = np.zeros((NCORES, T), dtype=np.int64)
        rel_s = np.full((NCORES, T), 255.0, dtype=np.float32)
        starts = np.concatenate([[0], np.cumsum(chunks)]) * P
        for c in range(NCORES):
            gstart = np.concatenate([[0], np.cumsum(cnts[c])])
            for g in range(ngroups):
                n = cnts[c][g]
                if n == 0:
                    continue
                s, d = gstart[g], starts[g]
                idx_s[c, d:d + n] = vals[c][s:s + n]
                rel_s[c, d:d + n] = relmod_keys[c][s:s + n] % P
        return idx_s, rel_s

    v2e_idx, v2e_rel = build_streams(v2e_lv, v2e_cnts, v2e_chunks, EG, v2e_e)
    e2v_idx, e2v_rel = build_streams(e2v_e, e2v_cnts, e2v_chunks, VG, e2v_lv)

    cnt = np.bincount(edge_idx, minlength=E_PAD).astype(np.float32)
    inv_cnt = 1.0 / np.maximum(cnt, 1.0)

    return dict(
        v2e_chunks=[int(x) for x in v2e_chunks],
        e2v_chunks=[int(x) for x in e2v_chunks],
        v2e_idx=v2e_idx, v2e_rel=v2e_rel,
        e2v_idx=e2v_idx, e2v_rel=e2v_rel,
        inv_cnt=inv_cnt,
    )


def _n_gather_calls(total_chunks, call_pairs):
    total = total_chunks * P
    n_full, rem = divmod(total, call_pairs)
    sizes = [call_pairs] * n_full
    if rem:
        sizes.append(rem)
    return sizes


def build_kernel(v2e_chunks, e2v_chunks, debug_tables=False):
    v2e_tot = sum(v2e_chunks) * P
    e2v_tot = sum(e2v_chunks) * P

    nc = bacc.Bacc("TRN2", target_bir_lowering=False, debug=False,
                   num_devices=NCORES, num_swdge_queues=2,
                   dynamic_dma_scratch_size=32768)

    x_in = nc.dram_tensor("x", [NPC_PAD, CH], F32, kind="ExternalInput")
    xt_in = nc.dram_tensor("xt", [P, VG * 2 * P], BF16, kind="ExternalInput")
    wcat_in = nc.dram_tensor("wcat", [CH, CH], BF16, kind="ExternalInput")
    convw_in = nc.dram_tensor("convw", [CH, CH], BF16, kind="ExternalInput")
    brep_in = nc.dram_tensor("brep", [P, CH], F32, kind="ExternalInput")
    awrep_in = nc.dram_tensor("awrep", [P, CH], BF16, kind="ExternalInput")
    convbrep_in = nc.dram_tensor("convbrep", [P, CH], F32, kind="ExternalInput")
    gammarep_in = nc.dram_tensor("gammarep", [P, CH], F32, kind="ExternalInput")
    lnwrep_in = nc.dram_tensor("lnwrep", [P, CH], F32, kind="ExternalInput")
    lnbrep_in = nc.dram_tensor("lnbrep", [P, CH], F32, kind="ExternalInput")
    iota_in = nc.dram_tensor("iota", [P, P], BF16, kind="ExternalInput")
    ident_in = nc.dram_tensor("ident", [P, P], BF16, kind="ExternalInput")
    invc_in = nc.dram_tensor("invc", [P, EG], F32, kind="ExternalInput")
    c14_in = nc.dram_tensor("c14", [P, H], F32, kind="ExternalInput")
    epscol_in = nc.dram_tensor("epscol", [P, 1], F32, kind="ExternalInput")
    v2ei_in = nc.dram_tensor("v2ei", [P, v2e_tot // 16], I16, kind="ExternalInput")
    v2er_in = nc.dram_tensor("v2er", [P, v2e_tot // P], BF16, kind="ExternalInput")
    e2vi_in = nc.dram_tensor("e2vi", [P, e2v_tot // 16], I16, kind="ExternalInput")
    e2vr_in = nc.dram_tensor("e2vr", [P, e2v_tot // P], BF16, kind="ExternalInput")
    out_ext = nc.dram_tensor("out", [NPC_PAD, CH], F32, kind="ExternalOutput")
    dbg = {}
    if debug_tables:
        dbg["xh"] = nc.dram_tensor("dbg_xh", [NPC_PAD, CH], BF16, kind="ExternalOutput")
        dbg["esum"] = nc.dram_tensor("dbg_esum", [E_PAD, CH], BF16, kind="ExternalOutput")
        dbg["z"] = nc.dram_tensor("dbg_z", [E_PAD, ZW], BF16, kind="ExternalOutput")
        dbg["xn"] = nc.dram_tensor("dbg_xn", [NPC_PAD, CH], BF16, kind="ExternalOutput")

    def rows(dr, t0, w):
        return dr[t0 * P:(t0 + w) * P, :].rearrange("(t p) f -> p t f", p=P)

    with tile.TileContext(nc) as tc:
        with tc.tile_pool(name="dram", bufs=1, space="DRAM") as dram, \
             tc.tile_pool(name="const", bufs=1) as cpool, \
             tc.tile_pool(name="resident", bufs=1) as rpool:

            nc.gpsimd.load_library(mlp)

            xh_table = dram.tile([NPC_PAD, CH], BF16)
            esum_bounce = dram.tile([E_PAD, CH], BF16)
            AR_BOUNDS = [40, 80, 120, EG]
            _ar_lims = list(zip([0] + AR_BOUNDS[:-1], AR_BOUNDS))
            yfulls = []
            for _ci, (_a, _b) in enumerate(_ar_lims):
                yf = dram.tile([(_b - _a) * P, CH], BF16, addr_space="Shared",
                               name=f"yfull{_ci}", tag=f"yfull{_ci}")
                yfulls.append(yf)
            z_table = dram.tile([E_PAD, ZW], BF16)

            def yrows(t0, w):
                """rows [t0*128,(t0+w)*128) of the chunked AR output; the
                caller must not cross an AR chunk boundary."""
                for (_a, _b), yf in zip(_ar_lims, yfulls):
                    if t0 >= _a and t0 + w <= _b:
                        return yf[(t0 - _a) * P:(t0 - _a + w) * P, :].rearrange(
                            "(t p) f -> p t f", p=P)
                raise AssertionError("yrows crosses AR chunk")

            def cload(dr, shape, dtype, name):
                t = cpool.tile(shape, dtype, name=name, tag=name)
                nc.sync.dma_start(t[:], dr[:])
                return t

            w_sb = cpool.tile([P, 2, CH], BF16)
            nc.sync.dma_start(w_sb[:], wcat_in[:].rearrange("(k p) f -> p k f", p=P))
            convw_sb = cpool.tile([P, 2, CH], BF16)
            nc.sync.dma_start(convw_sb[:], convw_in[:].rearrange("(k p) f -> p k f", p=P))
            brep = cload(brep_in, [P, CH], F32, "brep")
            awrep = cload(awrep_in, [P, CH], BF16, "awrep")
            convbrep = cload(convbrep_in, [P, CH], F32, "convbrep")
            gammarep = cload(gammarep_in, [P, CH], F32, "gammarep")
            lnwrep = cload(lnwrep_in, [P, CH], F32, "lnwrep")
            lnbrep = cload(lnbrep_in, [P, CH], F32, "lnbrep")
            iota = cload(iota_in, [P, P], BF16, "iota")
            ident = cload(ident_in, [P, P], BF16, "ident")
            invc = cload(invc_in, [P, EG], F32, "invc")
            c14 = cload(c14_in, [P, H], F32, "c14")
            epscol = cload(epscol_in, [P, 1], F32, "epscol")
            gcb = cpool.tile([P, CH], F32)
            nc.vector.tensor_tensor(out=gcb[:], in0=gammarep[:], in1=convbrep[:],
                                    op=AL.mult)
            # fold gamma into conv_w columns: (Xg @ W) * gamma = Xg @ (W * gamma_row)
            nc.vector.tensor_tensor(
                out=convw_sb[:], in0=convw_sb[:],
                in1=gammarep[:, None, :].to_broadcast([P, 2, CH]), op=AL.mult)

            xn_sb = rpool.tile([P, VG, CH], BF16)

            # ================= Phase 1: Xh = X @ W + b =================
            with tc.tile_pool(name="p1sb", bufs=3) as p1sb, \
                 tc.tile_pool(name="p1xt", bufs=1) as p1xt, \
                 tc.tile_pool(name="p1ps", bufs=2, space="PSUM") as p1ps:
                xt_sb = p1xt.tile([P, VG * 2 * P], BF16, tag="xt")
                nc.sync.dma_start(xt_sb[:], xt_in[:])
                xt_v = xt_sb[:].rearrange("p (t k f) -> p t k f", t=VG, k=2)
                xh4, t0, tw = None, 0, 0
                for t in range(VG):
                    psf = p1ps.tile([P, 512], F32, tag="xhps")
                    ps = psf[:, :CH]
                    for k in range(2):
                        nc.tensor.matmul(ps, lhsT=xt_v[:, t, k, :], rhs=w_sb[:, k, :],
                                         start=(k == 0), stop=(k == 1))
                    if t % 4 == 0:
                        t0 = t
                        tw = min(4, VG - t0)
                        xh4 = p1sb.tile([P, 4, CH], BF16, tag="xhout")
                    nc.vector.tensor_tensor(out=xh4[:, t - t0, :], in0=ps,
                                            in1=brep[:], op=AL.add)
                    if t - t0 == tw - 1:
                        nc.sync.dma_start(rows(xh_table, t0, tw), xh4[:, :tw, :])
                if debug_tables:
                    nc.sync.dma_start(dbg["xh"][:], xh_table[:])

            # ================= Phase 2: v2e partial esum =================
            with tc.tile_pool(name="v2esb", bufs=4) as gpool, \
                 tc.tile_pool(name="v2esel", bufs=3) as selpool, \
                 tc.tile_pool(name="v2eev", bufs=3) as evpool, \
                 tc.tile_pool(name="v2eidx", bufs=1) as ipool, \
                 tc.tile_pool(name="zsb", bufs=3) as zpool, \
                 tc.tile_pool(name="v2eps", bufs=3, space="PSUM") as v2eps:
                v2ei = ipool.tile([P, v2e_tot // 16], I16)
                nc.sync.dma_start(v2ei[:], v2ei_in[:])
                v2er = ipool.tile([P, v2e_tot // P], BF16)
                nc.sync.dma_start(v2er[:], v2er_in[:])

                def z_block(t_lo, t_hi):
                    for t0 in range(t_lo, t_hi, 4):
                        w = min(4, t_hi - t0)
                        y4 = zpool.tile([P, 4, CH], BF16, tag="zy")
                        nc.sync.dma_start(y4[:, :w, :], yrows(t0, w))
                        tmp = zpool.tile([P, 4, CH], BF16, tag="ztmp")
                        nc.vector.tensor_tensor(
                            out=tmp[:, :w, :], in0=y4[:, :w, :],
                            in1=awrep[:, None, :].to_broadcast([P, w, CH]), op=AL.mult)
                        beta = zpool.tile([P, 4, H], F32, tag="zbeta")
                        nc.vector.tensor_reduce(
                            out=beta[:, :w, :],
                            in_=tmp[:, :w, :].rearrange("p t (h d) -> p t h d", d=DH),
                            axis=mybir.AxisListType.X, op=AL.add)
                        al_ = zpool.tile([P, 4, H], F32, tag="zal")
                        nc.vector.tensor_tensor(
                            out=al_[:, :w, :], in0=beta[:, :w, :],
                            in1=invc[:, t0:t0 + w, None].to_broadcast([P, w, H]),
                            op=AL.mult)
                        sal = zpool.tile([P, 4, H], F32, tag="zsal")
                        nc.scalar.activation(out=sal[:, :w, :], in_=al_[:, :w, :],
                                             func=AF.Prelu, alpha=NEG_SLOPE)
                        zrow = zpool.tile([P, 4, CH + H], BF16, tag="zrow")
                        expS = zrow[:, :w, CH:CH + H]
                        nc.scalar.activation(out=expS, in_=sal[:, :w, :], func=AF.Exp)
                        s4 = zpool.tile([P, 4, H], F32, tag="zs4")
                        nc.vector.tensor_tensor(
                            out=s4[:, :w, :], in0=expS,
                            in1=invc[:, t0:t0 + w, None].to_broadcast([P, w, H]),
                            op=AL.mult)
                        nc.vector.tensor_tensor(
                            out=zrow[:, :w, :CH].rearrange("p t (h d) -> p t h d", d=DH),
                            in0=y4[:, :w, :].rearrange("p t (h d) -> p t h d", d=DH),
                            in1=s4[:, :w, :, None].to_broadcast([P, w, H, DH]),
                            op=AL.mult)
                        nc.sync.dma_start(
                            z_table[t0 * P:(t0 + w) * P, :CH + H].rearrange(
                                "(t p) f -> p t f", p=P),
                            zrow[:, :w, :])

                call_sizes = _n_gather_calls(sum(v2e_chunks), GATHER_CALL_V2E)
                gtiles = [None] * len(call_sizes)
                tot = sum(v2e_chunks)
                sel_cur, sel0 = None, 0
                esb4, e0, ew = None, 0, 0
                mm = 0
                for g in range(EG):
                    psf = v2eps.tile([P, 512], F32, tag="v2eps")
                    ps = psf[:, :CH]
                    for k in range(v2e_chunks[g]):
                        gc, j = divmod(mm, GATHER_CALL_V2E // P)
                        if gtiles[gc] is None:
                            n = call_sizes[gc]
                            gt = gpool.tile([P, GATHER_CALL_V2E // P, CH], BF16,
                                            tag="v2egather")
                            s = gc * GATHER_CALL_V2E
                            nc.gpsimd.dma_gather(
                                gt[:, :n // P, :], xh_table[:],
                                v2ei[:, s // 16:(s + n) // 16], n, n, CH,
                                single_packet=False, queue_num=gc % 2)
                            gtiles[gc] = gt
                        if mm % SELW == 0:
                            sel0 = mm
                            sw = min(SELW, tot - mm)
                            sel_cur = selpool.tile([P, SELW, P], BF16, tag="v2esel")
                            nc.vector.tensor_tensor(
                                out=sel_cur[:, :sw, :],
                                in0=v2er[:, mm:mm + sw, None].to_broadcast([P, sw, P]),
                                in1=iota[:, None, :].to_broadcast([P, sw, P]),
                                op=AL.is_equal)
                        nc.tensor.matmul(ps, lhsT=sel_cur[:, mm - sel0, :],
                                         rhs=gtiles[gc][:, j, :],
                                         start=(k == 0), stop=(k == v2e_chunks[g] - 1))
                        mm += 1
                    if g % 4 == 0:
                        e0 = g
                        ew = min(4, EG - e0)
                        esb4 = evpool.tile([P, 4, CH], BF16, tag="v2eev")
                    nc.vector.tensor_copy(out=esb4[:, g - e0, :], in_=ps)
                    if g - e0 == ew - 1:
                        nc.sync.dma_start(rows(esum_bounce, e0, ew), esb4[:, :ew, :])
                    if g + 1 in AR_BOUNDS:
                        ci = AR_BOUNDS.index(g + 1)
                        a = 0 if ci == 0 else AR_BOUNDS[ci - 1]
                        nc.gpsimd.collective_compute(
                            "AllReduce", AL.add,
                            replica_groups=[list(range(NCORES))],
                            ins=[esum_bounce[a * P:(g + 1) * P, :].opt()],
                            outs=[yfulls[ci].opt()])
                        z_block(a, g + 1)

            # ================= Phase 3: AllReduce (issued chunked in phase 2)
            if debug_tables:
                for (_a, _b), yf in zip(_ar_lims, yfulls):
                    nc.sync.dma_start(dbg["esum"][_a * P:_b * P, :], yf[:])

            if debug_tables:
                with tc.tile_pool(name="dbgz", bufs=1) as _dzp:
                    nc.sync.dma_start(dbg["z"][:], z_table[:])

            # ================= Phase 5: e2v + ELU + LN =================
            with tc.tile_pool(name="e2vsb", bufs=3) as gpool2, \
                 tc.tile_pool(name="e2vsel", bufs=3) as selpool2, \
                 tc.tile_pool(name="e2vev", bufs=2) as evpool2, \
                 tc.tile_pool(name="e2vidx", bufs=1) as ipool2, \
                 tc.tile_pool(name="e2vps", bufs=3, space="PSUM") as e2vps:
                e2vi = ipool2.tile([P, e2v_tot // 16], I16)
                nc.sync.dma_start(e2vi[:], e2vi_in[:])
                e2vr = ipool2.tile([P, e2v_tot // P], BF16)
                nc.sync.dma_start(e2vr[:], e2vr_in[:])

                call_sizes = _n_gather_calls(sum(e2v_chunks), GATHER_CALL_E2V)
                gtiles = [None] * len(call_sizes)
                tot = sum(e2v_chunks)
                sel_cur, sel0 = None, 0
                elu4, l0, lw = None, 0, 0
                mm = 0
                for g in range(VG):
                    psf = e2vps.tile([P, 512], F32, tag="e2vps")
                    ps = psf[:, :CH + H]
                    for k in range(e2v_chunks[g]):
                        gc, j = divmod(mm, GATHER_CALL_E2V // P)
                        if gtiles[gc] is None:
                            n = call_sizes[gc]
                            gt = gpool2.tile([P, GATHER_CALL_E2V // P, ZW], BF16,
                                             tag="e2vgather")
                            s = gc * GATHER_CALL_E2V
                            nc.gpsimd.dma_gather(
                                gt[:, :n // P, :], z_table[:],
                                e2vi[:, s // 16:(s + n) // 16], n, n, ZW,
                                single_packet=False, queue_num=gc % 2)
                            gtiles[gc] = gt
                        if mm % SELW == 0:
                            sel0 = mm
                            sw = min(SELW, tot - mm)
                            sel_cur = selpool2.tile([P, SELW, P], BF16, tag="e2vsel")
                            nc.vector.tensor_tensor(
                                out=sel_cur[:, :sw, :],
                                in0=e2vr[:, mm:mm + sw, None].to_broadcast([P, sw, P]),
                                in1=iota[:, None, :].to_broadcast([P, sw, P]),
                                op=AL.is_equal)
                        nc.tensor.matmul(ps, lhsT=sel_cur[:, mm - sel0, :],
                                         rhs=gtiles[gc][:, j, :CH + H],
                                         start=(k == 0), stop=(k == e2v_chunks[g] - 1))
                        mm += 1
                    # xpre = num/den ; ELU = exp(min(x,0)) - 1 + relu(x)
                    den = evpool2.tile([P, H], F32, tag="den")
                    nc.vector.tensor_scalar_max(den[:], ps[:, CH:CH + H], 1e-12)
                    rec = evpool2.tile([P, H], F32, tag="rec")
                    nc.vector.reciprocal(rec[:], den[:])
                    xpre = evpool2.tile([P, CH], BF16, tag="xpre")
                    nc.vector.tensor_tensor(
                        out=xpre[:].rearrange("p (h d) -> p h d", d=DH),
                        in0=ps[:, :CH].rearrange("p (h d) -> p h d", d=DH),
                        in1=rec[:, :, None].to_broadcast([P, H, DH]),
                        op=AL.mult)
                    relx = evpool2.tile([P, CH], BF16, tag="relx")
                    nc.scalar.activation(out=relx[:], in_=xpre[:], func=AF.Relu)
                    m0 = evpool2.tile([P, CH], BF16, tag="m0")
                    nc.scalar.activation(out=m0[:], in_=xpre[:], func=AF.Relu,
                                         scale=-1.0)
                    ep = evpool2.tile([P, CH], F32, tag="ep")
                    nc.scalar.activation(out=ep[:], in_=m0[:], func=AF.Exp,
                                         scale=-1.0)
                    if g % 4 == 0:
                        l0 = g
                        lw = min(4, VG - l0)
                        elu4 = evpool2.tile([P, 4, CH], BF16, tag="elu4")
                    nc.vector.scalar_tensor_tensor(
                        out=elu4[:, g - l0, :], in0=ep[:], scalar=-1.0, in1=relx[:],
                        op0=AL.add, op1=AL.add)
                    if g - l0 == lw - 1:
                        mu4 = evpool2.tile([P, 4], F32, tag="mu4")
                        nc.vector.tensor_reduce(out=mu4[:, :lw], in_=elu4[:, :lw, :],
                                                axis=mybir.AxisListType.X, op=AL.add)
                        nc.vector.tensor_tensor(out=mu4[:, :lw], in0=mu4[:, :lw],
                                                in1=c14[:, :lw], op=AL.mult)
                        xc4 = evpool2.tile([P, 4, CH], BF16, tag="xc4")
                        nc.vector.tensor_tensor(
                            out=xc4[:, :lw, :], in0=elu4[:, :lw, :],
                            in1=mu4[:, :lw, None].to_broadcast([P, lw, CH]),
                            op=AL.subtract)
                        sq4 = evpool2.tile([P, 4, CH], BF16, tag="sq4")
                        nc.vector.tensor_tensor(out=sq4[:, :lw, :], in0=xc4[:, :lw, :],
                                                in1=xc4[:, :lw, :], op=AL.mult)
                        ss4 = evpool2.tile([P, 4], F32, tag="ss4")
                        nc.vector.tensor_reduce(out=ss4[:, :lw], in_=sq4[:, :lw, :],
                                                axis=mybir.AxisListType.X, op=AL.add)
                        var4 = evpool2.tile([P, 4], F32, tag="var4")
                        nc.vector.tensor_tensor(out=var4[:, :lw], in0=ss4[:, :lw],
                                                in1=c14[:, :lw], op=AL.mult)
                        lnv4 = evpool2.tile([P, 4], F32, tag="lnv4")
                        nc.scalar.activation(out=lnv4[:, :lw], in_=var4[:, :lw],
                                             func=AF.Ln, bias=epscol[:])
                        rstd4 = evpool2.tile([P, 4], F32, tag="rstd4")
                        nc.scalar.activation(out=rstd4[:, :lw], in_=lnv4[:, :lw],
                                             func=AF.Exp, scale=-0.5)
                        t4 = evpool2.tile([P, 4, CH], BF16, tag="t4")
                        nc.vector.tensor_tensor(
                            out=t4[:, :lw, :], in0=xc4[:, :lw, :],
                            in1=rstd4[:, :lw, None].to_broadcast([P, lw, CH]),
                            op=AL.mult)
                        t5 = evpool2.tile([P, 4, CH], F32, tag="t5")
                        nc.vector.tensor_tensor(
                            out=t5[:, :lw, :], in0=t4[:, :lw, :],
                            in1=lnwrep[:, None, :].to_broadcast([P, lw, CH]),
                            op=AL.mult)
                        nc.vector.tensor_tensor(
                            out=xn_sb[:, l0:l0 + lw, :], in0=t5[:, :lw, :],
                            in1=lnbrep[:, None, :].to_broadcast([P, lw, CH]),
                            op=AL.add)

            if debug_tables:
                with tc.tile_pool(name="dbgxn", bufs=1) as dxp:
                    dx = dxp.tile([P, VG, CH], BF16)
                    nc.vector.tensor_copy(out=dx[:], in_=xn_sb[:])
                    nc.sync.dma_start(dbg["xn"][:].rearrange("(t p) f -> p t f", p=P),
                                      dx[:])

            # ================= Phase 6: GELU + conv + residual =============
            with tc.tile_pool(name="fsb", bufs=3) as fpool, \
                 tc.tile_pool(name="fps", bufs=2, space="PSUM") as fps, \
                 tc.tile_pool(name="ftps", bufs=4, space="PSUM") as ftps:
                for g0 in range(0, VG, 4):
                    w4 = min(4, VG - g0)
                    xg4 = fpool.tile([P, 4, CH], BF16, tag="xg4")
                    nc.scalar.activation(out=xg4[:, :w4, :],
                                         in_=xn_sb[:, g0:g0 + w4, :], func=AF.Gelu)
                    x4 = fpool.tile([P, 4, CH], F32, tag="x4")
                    nc.sync.dma_start(x4[:, :w4, :], rows(x_in, g0, w4))
                    xgc4 = fpool.tile([P, 4, CH], F32, tag="xgc4")
                    nc.vector.tensor_tensor(
                        out=xgc4[:, :w4, :], in0=x4[:, :w4, :],
                        in1=gcb[:, None, :].to_broadcast([P, w4, CH]), op=AL.add)
                    ofin4 = fpool.tile([P, 4, CH], F32, tag="ofin4")
                    for j in range(w4):
                        xgT = fpool.tile([P, 2, P], BF16, tag="xgT")
                        for k in range(2):
                            tp = ftps.tile([P, P], BF16, tag="tps")
                            nc.tensor.transpose(tp[:], xg4[:, j, k * P:(k + 1) * P],
                                                ident[:])
                            nc.scalar.copy(out=xgT[:, k, :], in_=tp[:])
                        psf = fps.tile([P, 512], F32, tag="fps")
                        ps = psf[:, :CH]
                        for k in range(2):
                            nc.tensor.matmul(ps, lhsT=xgT[:, k, :],
                                             rhs=convw_sb[:, k, :],
                                             start=(k == 0), stop=(k == 1))
                        nc.vector.tensor_tensor(out=ofin4[:, j, :], in0=ps,
                                                in1=xgc4[:, j, :], op=AL.add)
                    nc.sync.dma_start(rows(out_ext, g0, w4), ofin4[:, :w4, :])

    nc.compile()
    return nc


def prepare_inputs(X, edge_idx, vertex_idx, theta_w, theta_b, atten_w,
                   ln_w, ln_b, conv_w, conv_b, gamma, plan):
    X = np.asarray(X, dtype=np.float32)
    theta_w = np.asarray(theta_w, dtype=np.float32)
    wcat = _bf(theta_w.transpose(1, 0, 2).reshape(CH, CH))
    brep = np.tile(np.asarray(theta_b, np.float32).reshape(1, CH), (P, 1))
    awrep = _bf(np.tile(np.asarray(atten_w, np.float32).reshape(1, CH), (P, 1)))
    convw = _bf(np.asarray(conv_w, np.float32))
    convbrep = np.tile(np.asarray(conv_b, np.float32).reshape(1, CH), (P, 1))
    gammarep = np.tile(np.asarray(gamma, np.float32).reshape(1, CH), (P, 1))
    lnwrep = np.tile(np.asarray(ln_w, np.float32).reshape(1, CH), (P, 1))
    lnbrep = np.tile(np.asarray(ln_b, np.float32).reshape(1, CH), (P, 1))
    iota = _bf(np.tile(np.arange(P, dtype=np.float32), (P, 1)))
    ident = _bf(np.eye(P, dtype=np.float32))
    invc = np.ascontiguousarray(
        plan["inv_cnt"].reshape(EG, P).T).astype(np.float32)
    c14 = np.full((P, H), 1.0 / CH, np.float32)
    epscol = np.full((P, 1), LN_EPS, np.float32)

    in_maps = []
    for c in range(NCORES):
        xc = np.zeros((NPC_PAD, CH), np.float32)
        xc[:NPC] = X[c * NPC:(c + 1) * NPC]
        xcb = _bf(xc)
        xt = np.ascontiguousarray(
            xcb.reshape(VG, P, 2, P).transpose(3, 0, 2, 1)).reshape(P, VG * 2 * P)
        in_maps.append(dict(
            x=xc, xt=_bf(xt), wcat=wcat, convw=convw,
            brep=brep.astype(np.float32), awrep=awrep,
            convbrep=convbrep.astype(np.float32),
            gammarep=gammarep.astype(np.float32),
            lnwrep=lnwrep.astype(np.float32), lnbrep=lnbrep.astype(np.float32),
            iota=iota, ident=ident, invc=invc, c14=c14, epscol=epscol,
            v2ei=_wrap16(plan["v2e_idx"][c]),
            v2er=_pairmajor(plan["v2e_rel"][c], ml_dtypes.bfloat16),
            e2vi=_wrap16(plan["e2v_idx"][c]),
            e2vr=_pairmajor(plan["e2v_rel"][c], ml_dtypes.bfloat16),
        ))
    return in_maps


_CACHE = {}


def kernel(X, edge_idx, vertex_idx, theta_w, theta_b, atten_w,
           ln_w, ln_b, conv_w, conv_b, gamma):
    debug_tables = bool(int(os.environ.get("GNN_DEBUG_TABLES", "0")))
    trace = bool(int(os.environ.get("GNN_TRACE", "0")))

    plan = make_plan(edge_idx, vertex_idx)
    key = (tuple(plan["v2e_chunks"]), tuple(plan["e2v_chunks"]), debug_tables)
    if key not in _CACHE:
        _CACHE[key] = build_kernel(plan["v2e_chunks"], plan["e2v_chunks"],
                                   debug_tables=debug_tables)
    nc = _CACHE[key]

    in_maps = prepare_inputs(X, edge_idx, vertex_idx, theta_w, theta_b,
                             atten_w, ln_w, ln_b, conv_w, conv_b, gamma, plan)
    res = run_bass_kernel_spmd(nc, in_maps, core_ids=list(range(NCORES)),
                               trace=trace)
    kernel.last_results = res
    out = np.concatenate(
        [np.asarray(res.results[c]["out"])[:NPC] for c in range(NCORES)], axis=0)
    return out.astype(np.float32)


# revision 20
# speedup vs baseline: 2.1737x; 1.1083x over previous
"""Trainium2 8-core Bass kernel for the UniGAT hypergraph attention block.

Algorithm (matches the jax reference numerically, up to bf16 rounding):
  1. Xh = X @ theta_cat + b          (per-core node shard, PE matmul)
  2. v2e: esum[e] = sum over incidence pairs (e,v) of Xh[v]
       - per-core partial over its node shard: dma_gather of Xh rows per
         pair (sorted by edge) + 0/1-indicator segment matmul on PE
       - AllReduce(esum) over the 8 cores
  3. Softmax folding: w = exp(s)/sum(exp(s)) exactly (the segment-max
     subtraction cancels; s = leaky_relu in [-0.5, 0.5] so exp is safe).
     Build per-edge table Z = [Y*expS | expS] where Y = esum*inv_cnt,
     expS[e,h] = exp(leaky_relu(inv_cnt*(esum @ aw_h))).
  4. e2v: plain 0/1 segment-sum of gathered Z rows per destination vertex
     (sorted by vertex) -> numerator (256 cols) and denominator (4 cols);
     divide per head.
  5. ELU -> LayerNorm -> GELU -> conv matmul -> X + gamma * Xo.

Sharding: nodes (and pairs grouped by destination vertex) across 8 cores;
weights and edge tables replicated; one AllReduce of esum is the only
collective.
"""

import os

import numpy as np
import ml_dtypes

import concourse.bass as bass
import concourse.bacc as bacc
import concourse.tile as tile
import concourse.mybir as mybir
from concourse.bass_utils import run_bass_kernel_spmd
from concourse.library_config import mlp

BF16 = mybir.dt.bfloat16
F32 = mybir.dt.float32
I16 = mybir.dt.int16
AL = mybir.AluOpType
AF = mybir.ActivationFunctionType

P = 128
NCORES = 8

N_NODES = 100000
N_EDGES = 20000
NNZ = 500000
CH = 256
H = 4
DH = 64
NEG_SLOPE = 0.2
LN_EPS = 1e-6

NPC = N_NODES // NCORES          # 12500
VG = (NPC + P - 1) // P          # 98
NPC_PAD = VG * P                 # 12544
EG = (N_EDGES + P - 1) // P      # 157
E_PAD = EG * P                   # 20096

ZW = 384                         # Z table row stride (260 used; %128 elems)
GATHER_CALL_V2E = 4096
GATHER_CALL_E2V = 4096
SELW = 16                        # indicator chunks built per DVE op


def _bf(x):
    return np.asarray(x, dtype=ml_dtypes.bfloat16)


def _wrap16(idx):
    """dma_gather index layout: index i -> [i % 16, i // 16], replicated x8."""
    assert idx.size % 16 == 0
    w = idx.reshape(-1, 16).T
    return np.ascontiguousarray(np.tile(w, (8, 1))).astype(np.int16)


def _pairmajor(vals, dtype):
    """pair i -> [i % 128, i // 128]."""
    assert vals.size % P == 0
    return np.ascontiguousarray(vals.reshape(-1, P).T).astype(dtype)


def make_plan(edge_idx, vertex_idx):
    """Host-side index preprocessing (graph structure only)."""
    edge_idx = np.asarray(edge_idx).astype(np.int64)
    vertex_idx = np.asarray(vertex_idx).astype(np.int64)
    core = vertex_idx // NPC
    lv = vertex_idx - core * NPC

    v2e_e, v2e_lv = [], []
    e2v_e, e2v_lv = [], []
    for c in range(NCORES):
        m = core == c
        e_c, lv_c = edge_idx[m], lv[m]
        o = np.argsort(e_c, kind="stable")
        v2e_e.append(e_c[o])
        v2e_lv.append(lv_c[o])
        o = np.argsort(lv_c, kind="stable")
        e2v_e.append(e_c[o])
        e2v_lv.append(lv_c[o])

    def group_counts(keys_list, ngroups):
        cnts = np.zeros((NCORES, ngroups), dtype=np.int64)
        for c in range(NCORES):
            cnts[c] = np.bincount(keys_list[c] // P, minlength=ngroups)
        chunks = np.maximum(1, -(-cnts.max(axis=0) // P))
        return cnts, chunks

    v2e_cnts, v2e_chunks = group_counts(v2e_e, EG)
    e2v_cnts, e2v_chunks = group_counts(e2v_lv, VG)

    def build_streams(vals, cnts, chunks, ngroups, relmod_keys):
        T = int(chunks.sum()) * P
        idx_s = np.zeros((NCORES, T), dtype=np.int64)
        rel_s = np.full((NCORES, T), 255.0, dtype=np.float32)
        starts = np.concatenate([[0], np.cumsum(chunks)]) * P
        for c in range(NCORES):
            gstart = np.concatenate([[0], np.cumsum(cnts[c])])
            for g in range(ngroups):
                n = cnts[c][g]
                if n == 0:
                    continue
                s, d = gstart[g], starts[g]
                idx_s[c, d:d + n] = vals[c][s:s + n]
                rel_s[c, d:d + n] = relmod_keys[c][s:s + n] % P
        return idx_s, rel_s

    v2e_idx, v2e_rel = build_streams(v2e_lv, v2e_cnts, v2e_chunks, EG, v2e_e)
    e2v_idx, e2v_rel = build_streams(e2v_e, e2v_cnts, e2v_chunks, VG, e2v_lv)

    cnt = np.bincount(edge_idx, minlength=E_PAD).astype(np.float32)
    inv_cnt = 1.0 / np.maximum(cnt, 1.0)

    return dict(
        v2e_chunks=[int(x) for x in v2e_chunks],
        e2v_chunks=[int(x) for x in e2v_chunks],
        v2e_idx=v2e_idx, v2e_rel=v2e_rel,
        e2v_idx=e2v_idx, e2v_rel=e2v_rel,
        inv_cnt=inv_cnt,
    )


def _n_gather_calls(total_chunks, call_pairs):
    total = total_chunks * P
    n_full, rem = divmod(total, call_pairs)
    sizes = [call_pairs] * n_full
    if rem:
        sizes.append(rem)
    return sizes


def build_kernel(v2e_chunks, e2v_chunks, debug_tables=False):
    v2e_tot = sum(v2e_chunks) * P
    e2v_tot = sum(e2v_chunks) * P

    nc = bacc.Bacc("TRN2", target_bir_lowering=False, debug=False,
                   num_devices=NCORES, num_swdge_queues=2,
                   dynamic_dma_scratch_size=32768)

    x_in = nc.dram_tensor("x", [NPC_PAD, CH], F32, kind="ExternalInput")
    xt_in = nc.dram_tensor("xt", [P, VG * 2 * P], BF16, kind="ExternalInput")
    wcat_in = nc.dram_tensor("wcat", [CH, CH], BF16, kind="ExternalInput")
    convw_in = nc.dram_tensor("convw", [CH, CH], BF16, kind="ExternalInput")
    brep_in = nc.dram_tensor("brep", [P, CH], F32, kind="ExternalInput")
    awrep_in = nc.dram_tensor("awrep", [P, CH], BF16, kind="ExternalInput")
    convbrep_in = nc.dram_tensor("convbrep", [P, CH], F32, kind="ExternalInput")
    gammarep_in = nc.dram_tensor("gammarep", [P, CH], F32, kind="ExternalInput")
    lnwrep_in = nc.dram_tensor("lnwrep", [P, CH], F32, kind="ExternalInput")
    lnbrep_in = nc.dram_tensor("lnbrep", [P, CH], F32, kind="ExternalInput")
    iota_in = nc.dram_tensor("iota", [P, P], BF16, kind="ExternalInput")
    ident_in = nc.dram_tensor("ident", [P, P], BF16, kind="ExternalInput")
    invc_in = nc.dram_tensor("invc", [P, EG], F32, kind="ExternalInput")
    c14_in = nc.dram_tensor("c14", [P, H], F32, kind="ExternalInput")
    epscol_in = nc.dram_tensor("epscol", [P, 1], F32, kind="ExternalInput")
    v2ei_in = nc.dram_tensor("v2ei", [P, v2e_tot // 16], I16, kind="ExternalInput")
    v2er_in = nc.dram_tensor("v2er", [P, v2e_tot // P], BF16, kind="ExternalInput")
    e2vi_in = nc.dram_tensor("e2vi", [P, e2v_tot // 16], I16, kind="ExternalInput")
    e2vr_in = nc.dram_tensor("e2vr", [P, e2v_tot // P], BF16, kind="ExternalInput")
    out_ext = nc.dram_tensor("out", [NPC_PAD, CH], F32, kind="ExternalOutput")
    dbg = {}
    if debug_tables:
        dbg["xh"] = nc.dram_tensor("dbg_xh", [NPC_PAD, CH], BF16, kind="ExternalOutput")
        dbg["esum"] = nc.dram_tensor("dbg_esum", [E_PAD, CH], BF16, kind="ExternalOutput")
        dbg["z"] = nc.dram_tensor("dbg_z", [E_PAD, ZW], BF16, kind="ExternalOutput")
        dbg["xn"] = nc.dram_tensor("dbg_xn", [NPC_PAD, CH], BF16, kind="ExternalOutput")

    def rows(dr, t0, w):
        return dr[t0 * P:(t0 + w) * P, :].rearrange("(t p) f -> p t f", p=P)

    with tile.TileContext(nc) as tc:
        with tc.tile_pool(name="dram", bufs=1, space="DRAM") as dram, \
             tc.tile_pool(name="const", bufs=1) as cpool, \
             tc.tile_pool(name="resident", bufs=1) as rpool:

            nc.gpsimd.load_library(mlp)

            xh_table = dram.tile([NPC_PAD, CH], BF16)
            esum_bounce = dram.tile([E_PAD, CH], BF16)
            AR_BOUNDS = [40, 80, 120, EG]
            _ar_lims = list(zip([0] + AR_BOUNDS[:-1], AR_BOUNDS))
            yfulls = []
            for _ci, (_a, _b) in enumerate(_ar_lims):
                yf = dram.tile([(_b - _a) * P, CH], BF16, addr_space="Shared",
                               name=f"yfull{_ci}", tag=f"yfull{_ci}")
                yfulls.append(yf)
            z_table = dram.tile([E_PAD, ZW], BF16)

            def yrows(t0, w):
                """rows [t0*128,(t0+w)*128) of the chunked AR output; the
                caller must not cross an AR chunk boundary."""
                for (_a, _b), yf in zip(_ar_lims, yfulls):
                    if t0 >= _a and t0 + w <= _b:
                        return yf[(t0 - _a) * P:(t0 - _a + w) * P, :].rearrange(
                            "(t p) f -> p t f", p=P)
                raise AssertionError("yrows crosses AR chunk")

            def cload(dr, shape, dtype, name):
                t = cpool.tile(shape, dtype, name=name, tag=name)
                nc.sync.dma_start(t[:], dr[:])
                return t

            w_sb = cpool.tile([P, 2, CH], BF16)
            nc.sync.dma_start(w_sb[:], wcat_in[:].rearrange("(k p) f -> p k f", p=P))
            convw_sb = cpool.tile([P, 2, CH], BF16)
            nc.sync.dma_start(convw_sb[:], convw_in[:].rearrange("(k p) f -> p k f", p=P))
            brep = cload(brep_in, [P, CH], F32, "brep")
            awrep = cload(awrep_in, [P, CH], BF16, "awrep")
            convbrep = cload(convbrep_in, [P, CH], F32, "convbrep")
            gammarep = cload(gammarep_in, [P, CH], F32, "gammarep")
            lnwrep = cload(lnwrep_in, [P, CH], F32, "lnwrep")
            lnbrep = cload(lnbrep_in, [P, CH], F32, "lnbrep")
            iota = cload(iota_in, [P, P], BF16, "iota")
            ident = cload(ident_in, [P, P], BF16, "ident")
            invc = cload(invc_in, [P, EG], F32, "invc")
            c14 = cload(c14_in, [P, H], F32, "c14")
            epscol = cload(epscol_in, [P, 1], F32, "epscol")
            gcb = cpool.tile([P, CH], F32)
            nc.vector.tensor_tensor(out=gcb[:], in0=gammarep[:], in1=convbrep[:],
                                    op=AL.mult)
            # fold gamma into conv_w columns: (Xg @ W) * gamma = Xg @ (W * gamma_row)
            nc.vector.tensor_tensor(
                out=convw_sb[:], in0=convw_sb[:],
                in1=gammarep[:, None, :].to_broadcast([P, 2, CH]), op=AL.mult)

            # ================= Phase 1: Xh = X @ W + b =================
            with tc.tile_pool(name="p1sb", bufs=3) as p1sb, \
                 tc.tile_pool(name="p1xt", bufs=1) as p1xt, \
                 tc.tile_pool(name="p1ps", bufs=2, space="PSUM") as p1ps:
                xt_sb = p1xt.tile([P, VG * 2 * P], BF16, tag="xt")
                nc.sync.dma_start(xt_sb[:], xt_in[:])
                xt_v = xt_sb[:].rearrange("p (t k f) -> p t k f", t=VG, k=2)
                xh4, t0, tw = None, 0, 0
                for t in range(VG):
                    psf = p1ps.tile([P, 512], F32, tag="xhps")
                    ps = psf[:, :CH]
                    for k in range(2):
                        nc.tensor.matmul(ps, lhsT=xt_v[:, t, k, :], rhs=w_sb[:, k, :],
                                         start=(k == 0), stop=(k == 1))
                    if t % 4 == 0:
                        t0 = t
                        tw = min(4, VG - t0)
                        xh4 = p1sb.tile([P, 4, CH], BF16, tag="xhout")
                    nc.vector.tensor_tensor(out=xh4[:, t - t0, :], in0=ps,
                                            in1=brep[:], op=AL.add)
                    if t - t0 == tw - 1:
                        nc.sync.dma_start(rows(xh_table, t0, tw), xh4[:, :tw, :])
                if debug_tables:
                    nc.sync.dma_start(dbg["xh"][:], xh_table[:])

            # ================= Phase 2: v2e partial esum =================
            with tc.tile_pool(name="v2esb", bufs=4) as gpool, \
                 tc.tile_pool(name="v2esel", bufs=3) as selpool, \
                 tc.tile_pool(name="v2eev", bufs=3) as evpool, \
                 tc.tile_pool(name="v2eidx", bufs=1) as ipool, \
                 tc.tile_pool(name="zsb", bufs=3) as zpool, \
                 tc.tile_pool(name="v2eps", bufs=3, space="PSUM") as v2eps:
                v2ei = ipool.tile([P, v2e_tot // 16], I16)
                nc.sync.dma_start(v2ei[:], v2ei_in[:])
                v2er = ipool.tile([P, v2e_tot // P], BF16)
                nc.sync.dma_start(v2er[:], v2er_in[:])

                def z_block(t_lo, t_hi):
                    for t0 in range(t_lo, t_hi, 4):
                        w = min(4, t_hi - t0)
                        y4 = zpool.tile([P, 4, CH], BF16, tag="zy")
                        nc.sync.dma_start(y4[:, :w, :], yrows(t0, w))
                        tmp = zpool.tile([P, 4, CH], BF16, tag="ztmp")
                        nc.vector.tensor_tensor(
                            out=tmp[:, :w, :], in0=y4[:, :w, :],
                            in1=awrep[:, None, :].to_broadcast([P, w, CH]), op=AL.mult)
                        beta = zpool.tile([P, 4, H], F32, tag="zbeta")
                        nc.vector.tensor_reduce(
                            out=beta[:, :w, :],
                            in_=tmp[:, :w, :].rearrange("p t (h d) -> p t h d", d=DH),
                            axis=mybir.AxisListType.X, op=AL.add)
                        al_ = zpool.tile([P, 4, H], F32, tag="zal")
                        nc.vector.tensor_tensor(
                            out=al_[:, :w, :], in0=beta[:, :w, :],
                            in1=invc[:, t0:t0 + w, None].to_broadcast([P, w, H]),
                            op=AL.mult)
                        sal = zpool.tile([P, 4, H], F32, tag="zsal")
                        nc.scalar.activation(out=sal[:, :w, :], in_=al_[:, :w, :],
                                             func=AF.Prelu, alpha=NEG_SLOPE)
                        zrow = zpool.tile([P, 4, CH + H], BF16, tag="zrow")
                        expS = zrow[:, :w, CH:CH + H]
                        nc.scalar.activation(out=expS, in_=sal[:, :w, :], func=AF.Exp)
                        s4 = zpool.tile([P, 4, H], F32, tag="zs4")
                        nc.vector.tensor_tensor(
                            out=s4[:, :w, :], in0=expS,
                            in1=invc[:, t0:t0 + w, None].to_broadcast([P, w, H]),
                            op=AL.mult)
                        nc.vector.tensor_tensor(
                            out=zrow[:, :w, :CH].rearrange("p t (h d) -> p t h d", d=DH),
                            in0=y4[:, :w, :].rearrange("p t (h d) -> p t h d", d=DH),
                            in1=s4[:, :w, :, None].to_broadcast([P, w, H, DH]),
                            op=AL.mult)
                        nc.sync.dma_start(
                            z_table[t0 * P:(t0 + w) * P, :CH + H].rearrange(
                                "(t p) f -> p t f", p=P),
                            zrow[:, :w, :])

                call_sizes = _n_gather_calls(sum(v2e_chunks), GATHER_CALL_V2E)
                gtiles = [None] * len(call_sizes)
                tot = sum(v2e_chunks)
                sel_cur, sel0 = None, 0
                esb4, e0, ew = None, 0, 0
                mm = 0
                _zq = {}
                for _ci, (_a, _b) in enumerate(_ar_lims):
                    _zq.setdefault(_b + 35, []).append((_a, _b))
                for g in range(EG):
                    for _a, _b in _zq.get(g, []):
                        z_blocks(zpool_v2e, _a, _b)
                    psf = v2eps.tile([P, 512], F32, tag="v2eps")
                    ps = psf[:, :CH]
                    for k in range(v2e_chunks[g]):
                        gc, j = divmod(mm, GATHER_CALL_V2E // P)
                        if gtiles[gc] is None:
                            n = call_sizes[gc]
                            gt = gpool.tile([P, GATHER_CALL_V2E // P, CH], BF16,
                                            tag="v2egather")
                            s = gc * GATHER_CALL_V2E
                            nc.gpsimd.dma_gather(
                                gt[:, :n // P, :], xh_table[:],
                                v2ei[:, s // 16:(s + n) // 16], n, n, CH,
                                single_packet=False, queue_num=gc % 2)
                            gtiles[gc] = gt
                        if mm % SELW == 0:
                            sel0 = mm
                            sw = min(SELW, tot - mm)
                            sel_cur = selpool.tile([P, SELW, P], BF16, tag="v2esel")
                            nc.vector.tensor_tensor(
                                out=sel_cur[:, :sw, :],
                                in0=v2er[:, mm:mm + sw, None].to_broadcast([P, sw, P]),
                                in1=iota[:, None, :].to_broadcast([P, sw, P]),
                                op=AL.is_equal)
                        nc.tensor.matmul(ps, lhsT=sel_cur[:, mm - sel0, :],
                                         rhs=gtiles[gc][:, j, :],
                                         start=(k == 0), stop=(k == v2e_chunks[g] - 1))
                        mm += 1
                    if g % 4 == 0:
                        e0 = g
                        ew = min(4, EG - e0)
                        esb4 = evpool.tile([P, 4, CH], BF16, tag="v2eev")
                    nc.vector.tensor_copy(out=esb4[:, g - e0, :], in_=ps)
                    if g - e0 == ew - 1:
                        nc.sync.dma_start(rows(esum_bounce, e0, ew), esb4[:, :ew, :])
                    if g + 1 in AR_BOUNDS:
                        ci = AR_BOUNDS.index(g + 1)
                        a = 0 if ci == 0 else AR_BOUNDS[ci - 1]
                        nc.gpsimd.collective_compute(
                            "AllReduce", AL.add,
                            replica_groups=[list(range(NCORES))],
                            ins=[esum_bounce[a * P:(g + 1) * P, :].opt()],
                            outs=[yfulls[ci].opt()])
                        z_block(a, g + 1)

                for g in range(EG, EG + 64):
                    for _a, _b in _zq.get(g, []):
                        z_blocks(zpool_v2e, _a, _b)

            # ================= Phase 3: AllReduce (issued chunked in phase 2)
            if debug_tables:
                for (_a, _b), yf in zip(_ar_lims, yfulls):
                    nc.sync.dma_start(dbg["esum"][_a * P:_b * P, :], yf[:])

            if debug_tables:
                with tc.tile_pool(name="dbgz", bufs=1) as _dzp:
                    nc.sync.dma_start(dbg["z"][:], z_table[:])

            # ================= Phase 5: e2v + ELU + LN =================
            with tc.tile_pool(name="e2vsb", bufs=3) as gpool2, \
                 tc.tile_pool(name="e2vsel", bufs=3) as selpool2, \
                 tc.tile_pool(name="e2vev", bufs=2) as evpool2, \
                 tc.tile_pool(name="e2vidx", bufs=1) as ipool2, \
                 tc.tile_pool(name="fsb", bufs=3) as fpool, \
                 tc.tile_pool(name="fps", bufs=2, space="PSUM") as fps, \
                 tc.tile_pool(name="ftps", bufs=2, space="PSUM") as ftps, \
                 tc.tile_pool(name="e2vps", bufs=3, space="PSUM") as e2vps:
                e2vi = ipool2.tile([P, e2v_tot // 16], I16)
                nc.sync.dma_start(e2vi[:], e2vi_in[:])
                e2vr = ipool2.tile([P, e2v_tot // P], BF16)
                nc.sync.dma_start(e2vr[:], e2vr_in[:])

                call_sizes = _n_gather_calls(sum(e2v_chunks), GATHER_CALL_E2V)
                gtiles = [None] * len(call_sizes)
                tot = sum(e2v_chunks)
                sel_cur, sel0 = None, 0
                elu4, l0, lw = None, 0, 0
                mm = 0
                for g in range(VG):
                    psf = e2vps.tile([P, 512], F32, tag="e2vps")
                    ps = psf[:, :CH + H]
                    for k in range(e2v_chunks[g]):
                        gc, j = divmod(mm, GATHER_CALL_E2V // P)
                        if gtiles[gc] is None:
                            n = call_sizes[gc]
                            gt = gpool2.tile([P, GATHER_CALL_E2V // P, ZW], BF16,
                                             tag="e2vgather")
                            s = gc * GATHER_CALL_E2V
                            nc.gpsimd.dma_gather(
                                gt[:, :n // P, :], z_table[:],
                                e2vi[:, s // 16:(s + n) // 16], n, n, ZW,
                                single_packet=False, queue_num=gc % 2)
                            gtiles[gc] = gt
                        if mm % SELW == 0:
                            sel0 = mm
                            sw = min(SELW, tot - mm)
                            sel_cur = selpool2.tile([P, SELW, P], BF16, tag="e2vsel")
                            nc.vector.tensor_tensor(
                                out=sel_cur[:, :sw, :],
                                in0=e2vr[:, mm:mm + sw, None].to_broadcast([P, sw, P]),
                                in1=iota[:, None, :].to_broadcast([P, sw, P]),
                                op=AL.is_equal)
                        nc.tensor.matmul(ps, lhsT=sel_cur[:, mm - sel0, :],
                                         rhs=gtiles[gc][:, j, :CH + H],
                                         start=(k == 0), stop=(k == e2v_chunks[g] - 1))
                        mm += 1
                    # xpre = num/den ; ELU = exp(min(x,0)) - 1 + relu(x)
                    den = evpool2.tile([P, H], F32, tag="den")
                    nc.vector.tensor_scalar_max(den[:], ps[:, CH:CH + H], 1e-12)
                    rec = evpool2.tile([P, H], F32, tag="rec")
                    nc.vector.reciprocal(rec[:], den[:])
                    xpre = evpool2.tile([P, CH], BF16, tag="xpre")
                    nc.vector.tensor_tensor(
                        out=xpre[:].rearrange("p (h d) -> p h d", d=DH),
                        in0=ps[:, :CH].rearrange("p (h d) -> p h d", d=DH),
                        in1=rec[:, :, None].to_broadcast([P, H, DH]),
                        op=AL.mult)
                    relx = evpool2.tile([P, CH], BF16, tag="relx")
                    nc.scalar.activation(out=relx[:], in_=xpre[:], func=AF.Relu)
                    m0 = evpool2.tile([P, CH], BF16, tag="m0")
                    nc.scalar.activation(out=m0[:], in_=xpre[:], func=AF.Relu,
                                         scale=-1.0)
                    ep = evpool2.tile([P, CH], F32, tag="ep")
                    nc.scalar.activation(out=ep[:], in_=m0[:], func=AF.Exp,
                                         scale=-1.0)
                    if g % 4 == 0:
                        l0 = g
                        lw = min(4, VG - l0)
                        elu4 = evpool2.tile([P, 4, CH], BF16, tag="elu4")
                    nc.vector.scalar_tensor_tensor(
                        out=elu4[:, g - l0, :], in0=ep[:], scalar=-1.0, in1=relx[:],
                        op0=AL.add, op1=AL.add)
                    if g - l0 == lw - 1:
                        mu4 = evpool2.tile([P, 4], F32, tag="mu4")
                        nc.vector.tensor_reduce(out=mu4[:, :lw], in_=elu4[:, :lw, :],
                                                axis=mybir.AxisListType.X, op=AL.add)
                        nc.vector.tensor_tensor(out=mu4[:, :lw], in0=mu4[:, :lw],
                                                in1=c14[:, :lw], op=AL.mult)
                        xc4 = evpool2.tile([P, 4, CH], BF16, tag="xc4")
                        nc.vector.tensor_tensor(
                            out=xc4[:, :lw, :], in0=elu4[:, :lw, :],
                            in1=mu4[:, :lw, None].to_broadcast([P, lw, CH]),
                            op=AL.subtract)
                        sq4 = evpool2.tile([P, 4, CH], BF16, tag="sq4")
                        nc.vector.tensor_tensor(out=sq4[:, :lw, :], in0=xc4[:, :lw, :],
                                                in1=xc4[:, :lw, :], op=AL.mult)
                        ss4 = evpool2.tile([P, 4], F32, tag="ss4")
                        nc.vector.tensor_reduce(out=ss4[:, :lw], in_=sq4[:, :lw, :],
                                                axis=mybir.AxisListType.X, op=AL.add)
                        var4 = evpool2.tile([P, 4], F32, tag="var4")
                        nc.vector.tensor_tensor(out=var4[:, :lw], in0=ss4[:, :lw],
                                                in1=c14[:, :lw], op=AL.mult)
                        lnv4 = evpool2.tile([P, 4], F32, tag="lnv4")
                        nc.scalar.activation(out=lnv4[:, :lw], in_=var4[:, :lw],
                                             func=AF.Ln, bias=epscol[:])
                        rstd4 = evpool2.tile([P, 4], F32, tag="rstd4")
                        nc.scalar.activation(out=rstd4[:, :lw], in_=lnv4[:, :lw],
                                             func=AF.Exp, scale=-0.5)
                        t4 = evpool2.tile([P, 4, CH], BF16, tag="t4")
                        nc.vector.tensor_tensor(
                            out=t4[:, :lw, :], in0=xc4[:, :lw, :],
                            in1=rstd4[:, :lw, None].to_broadcast([P, lw, CH]),
                            op=AL.mult)
                        t5 = evpool2.tile([P, 4, CH], F32, tag="t5")
                        nc.vector.tensor_tensor(
                            out=t5[:, :lw, :], in0=t4[:, :lw, :],
                            in1=lnwrep[:, None, :].to_broadcast([P, lw, CH]),
                            op=AL.mult)
                        xnb = evpool2.tile([P, 4, CH], BF16, tag="xnb")
                        nc.vector.tensor_tensor(
                            out=xnb[:, :lw, :], in0=t5[:, :lw, :],
                            in1=lnbrep[:, None, :].to_broadcast([P, lw, CH]),
                            op=AL.add)
                        if debug_tables:
                            nc.sync.dma_start(rows(dbg["xn"], l0, lw),
                                              xnb[:, :lw, :])
                        # ---- fused final block: GELU + conv + residual ----
                        xg4 = fpool.tile([P, 4, CH], BF16, tag="xg4")
                        nc.scalar.activation(out=xg4[:, :lw, :], in_=xnb[:, :lw, :],
                                             func=AF.Gelu)
                        x4 = fpool.tile([P, 4, CH], F32, tag="x4")
                        nc.sync.dma_start(x4[:, :lw, :], rows(x_in, l0, lw))
                        xgc4 = fpool.tile([P, 4, CH], F32, tag="xgc4")
                        nc.vector.tensor_tensor(
                            out=xgc4[:, :lw, :], in0=x4[:, :lw, :],
                            in1=gcb[:, None, :].to_broadcast([P, lw, CH]), op=AL.add)
                        ofin4 = fpool.tile([P, 4, CH], F32, tag="ofin4")
                        for j in range(lw):
                            xgT = fpool.tile([P, 2, P], BF16, tag="xgT")
                            for k in range(2):
                                tp = ftps.tile([P, P], BF16, tag="tps")
                                nc.tensor.transpose(tp[:], xg4[:, j, k * P:(k + 1) * P],
                                                    ident[:])
                                nc.scalar.copy(out=xgT[:, k, :], in_=tp[:])
                            psf2 = fps.tile([P, 512], F32, tag="fps")
                            ps2 = psf2[:, :CH]
                            for k in range(2):
                                nc.tensor.matmul(ps2, lhsT=xgT[:, k, :],
                                                 rhs=convw_sb[:, k, :],
                                                 start=(k == 0), stop=(k == 1))
                            nc.vector.tensor_tensor(out=ofin4[:, j, :], in0=ps2,
                                                    in1=xgc4[:, j, :], op=AL.add)
                        nc.sync.dma_start(rows(out_ext, l0, lw), ofin4[:, :lw, :])

    nc.compile()
    return nc


def prepare_inputs(X, edge_idx, vertex_idx, theta_w, theta_b, atten_w,
                   ln_w, ln_b, conv_w, conv_b, gamma, plan):
    X = np.asarray(X, dtype=np.float32)
    theta_w = np.asarray(theta_w, dtype=np.float32)
    wcat = _bf(theta_w.transpose(1, 0, 2).reshape(CH, CH))
    brep = np.tile(np.asarray(theta_b, np.float32).reshape(1, CH), (P, 1))
    awrep = _bf(np.tile(np.asarray(atten_w, np.float32).reshape(1, CH), (P, 1)))
    convw = _bf(np.asarray(conv_w, np.float32))
    convbrep = np.tile(np.asarray(conv_b, np.float32).reshape(1, CH), (P, 1))
    gammarep = np.tile(np.asarray(gamma, np.float32).reshape(1, CH), (P, 1))
    lnwrep = np.tile(np.asarray(ln_w, np.float32).reshape(1, CH), (P, 1))
    lnbrep = np.tile(np.asarray(ln_b, np.float32).reshape(1, CH), (P, 1))
    iota = _bf(np.tile(np.arange(P, dtype=np.float32), (P, 1)))
    ident = _bf(np.eye(P, dtype=np.float32))
    invc = np.ascontiguousarray(
        plan["inv_cnt"].reshape(EG, P).T).astype(np.float32)
    c14 = np.full((P, H), 1.0 / CH, np.float32)
    epscol = np.full((P, 1), LN_EPS, np.float32)

    in_maps = []
    for c in range(NCORES):
        xc = np.zeros((NPC_PAD, CH), np.float32)
        xc[:NPC] = X[c * NPC:(c + 1) * NPC]
        xcb = _bf(xc)
        xt = np.ascontiguousarray(
            xcb.reshape(VG, P, 2, P).transpose(3, 0, 2, 1)).reshape(P, VG * 2 * P)
        in_maps.append(dict(
            x=xc, xt=_bf(xt), wcat=wcat, convw=convw,
            brep=brep.astype(np.float32), awrep=awrep,
            convbrep=convbrep.astype(np.float32),
            gammarep=gammarep.astype(np.float32),
            lnwrep=lnwrep.astype(np.float32), lnbrep=lnbrep.astype(np.float32),
            iota=iota, ident=ident, invc=invc, c14=c14, epscol=epscol,
            v2ei=_wrap16(plan["v2e_idx"][c]),
            v2er=_pairmajor(plan["v2e_rel"][c], ml_dtypes.bfloat16),
            e2vi=_wrap16(plan["e2v_idx"][c]),
            e2vr=_pairmajor(plan["e2v_rel"][c], ml_dtypes.bfloat16),
        ))
    return in_maps


_CACHE = {}


def kernel(X, edge_idx, vertex_idx, theta_w, theta_b, atten_w,
           ln_w, ln_b, conv_w, conv_b, gamma):
    debug_tables = bool(int(os.environ.get("GNN_DEBUG_TABLES", "0")))
    trace = bool(int(os.environ.get("GNN_TRACE", "0")))

    plan = make_plan(edge_idx, vertex_idx)
    key = (tuple(plan["v2e_chunks"]), tuple(plan["e2v_chunks"]), debug_tables)
    if key not in _CACHE:
        _CACHE[key] = build_kernel(plan["v2e_chunks"], plan["e2v_chunks"],
                                   debug_tables=debug_tables)
    nc = _CACHE[key]

    in_maps = prepare_inputs(X, edge_idx, vertex_idx, theta_w, theta_b,
                             atten_w, ln_w, ln_b, conv_w, conv_b, gamma, plan)
    res = run_bass_kernel_spmd(nc, in_maps, core_ids=list(range(NCORES)),
                               trace=trace)
    kernel.last_results = res
    out = np.concatenate(
        [np.asarray(res.results[c]["out"])[:NPC] for c in range(NCORES)], axis=0)
    return out.astype(np.float32)
